# revision 16
# baseline (speedup 1.0000x reference)
"""Llama GQA attention block, tensor-parallel over heads across 8 TRN2 NeuronCores.

Contract: kernel(**inputs) takes the FULL inputs of the reference
(x, freq_cos, freq_sin, w_q_w, w_q_b, w_kv_w, w_kv_b, proj_w, proj_b, start_pos)
and returns the FULL output (B, T, N_EMBD) float32.

Sharding: core c owns query heads 4c..4c+3 and KV head c, plus proj rows
c*512..(c+1)*512. Each core computes a partial projection output (fp16); the
host sums the 8 partials and adds proj_b.

v2 design (vs the 920us baseline):
- softmax row sums no longer use a per-kc ones-matmul on the PE (was 164K
  cycles/core).  exp tiles accumulate into a bf16 SBUF accumulator on the DVE
  (2x packed mode); a single 4-column selector matmul per (b,j) then reduces
  the 128 key-residues, yielding a [4,512] sums tile (one bank, all 4 heads).
- causal trimming: in diagonal 512-blocks the score/AV matmuls and the DVE
  accumulation only cover the valid query range (moving-dim slice); only the
  leading 128-wide triangle needs a mask multiply (GPSIMD, [128,128]).
- both heads of a pair share one 2-bank PSUM score tile [128,1024]; exp runs
  as a single ACTIVATE over 1024 columns (amortizes the ~293ns ACT overhead).
- x is loaded in 4-e-chunk granules (one 512KB DMA for [128,4,512]) and
  output tiles are stored 4-at-a-time (512KB DMAs), cutting HWDGE queue
  dispatch time ~4x and removing the pass-1 chunk-boundary DMA bubbles.
- out-projection stays a PE filler stream popped during attention (exp on the
  ACT engine is the per-kc critical path; filler keeps the PE saturated).
"""

import math
import numpy as np
from contextlib import ExitStack

# Problem constants (hardcoded per the harness contract).
B = 2
T = 2048
E = 4096
D = 128          # head dim
NCORES = 8
HPC = 4          # query heads per core
BT = B * T       # 4096
SQ = 512         # token chunk (matmul moving dim)
ECH = E // 128   # 32 contraction chunks
GE = 4           # e-chunks per x/weight DMA granule
NG = ECH // GE   # granules per token chunk
CPB = T // SQ    # 4 tok chunks per batch
INV_SQRT_D = 1.0 / math.sqrt(D)
HB = 1024        # half-batch token span for the output projection stage


def _build_program():
    import concourse.bass as bass  # noqa: F401
    import concourse.bass_isa as bass_isa
    import concourse.mybir as mybir
    import concourse.tile as tile
    from concourse import bacc

    f32 = mybir.dt.float32
    bf16 = mybir.dt.bfloat16
    f16 = mybir.dt.float16
    AF = mybir.ActivationFunctionType

    nc = bacc.Bacc("TRN2", target_bir_lowering=False, debug=False)

    xT_d = nc.dram_tensor("xT", [E, BT], bf16, kind="ExternalInput")
    wq_d = nc.dram_tensor("wqT", [E, HPC * D], bf16, kind="ExternalInput")
    wkv_d = nc.dram_tensor("wkvT", [E, 2 * D], bf16, kind="ExternalInput")
    bias_d = nc.dram_tensor("biases", [6, 128], f32, kind="ExternalInput")
    cos_d = nc.dram_tensor("cosE", [128, T], bf16, kind="ExternalInput")
    sin_d = nc.dram_tensor("sinS", [128, T], bf16, kind="ExternalInput")
    tri_d = nc.dram_tensor("triM", [128, 128], bf16, kind="ExternalInput")
    pjt_d = nc.dram_tensor("projT", [HPC * D, E], bf16, kind="ExternalInput")
    idn_d = nc.dram_tensor("ident", [128, 128], f32, kind="ExternalInput")
    out_d = nc.dram_tensor("yp", [BT, E], f16, kind="ExternalOutput")

    with tile.TileContext(nc) as tc, ExitStack() as ctx:
        const = ctx.enter_context(tc.tile_pool(name="const", bufs=1))
        wpool = ctx.enter_context(tc.tile_pool(name="wpool", bufs=1))
        big = ctx.enter_context(tc.tile_pool(name="big", bufs=1))
        xpool = ctx.enter_context(tc.tile_pool(name="xpool", bufs=3))
        espool = ctx.enter_context(tc.tile_pool(name="espool", bufs=3))
        accp = ctx.enter_context(tc.tile_pool(name="accp", bufs=2))
        spool = ctx.enter_context(tc.tile_pool(name="spool", bufs=2))
        ppool = ctx.enter_context(tc.tile_pool(name="ppool", bufs=2))
        ypool = ctx.enter_context(tc.tile_pool(name="ypool", bufs=2))
        psum = ctx.enter_context(tc.tile_pool(name="ps", bufs=1, space="PSUM"))

        # ---- weights / constants resident in SBUF ----
        wq_sb = wpool.tile([128, ECH, HPC * D], bf16, tag="wq")
        wkv_sb = wpool.tile([128, ECH, 2 * D], bf16, tag="wkv")
        # granule loads on the scalar HWDGE ring (x rides the sync ring).
        # wq/wkv granules interleave so the first k/v matmuls aren't stuck
        # behind all 4MB of wq; the leading granules are small for fast start.
        WGRAN = [(0, 1), (1, 1), (2, 2), (4, 4)]
        for e0, ge in WGRAN:
            nc.scalar.dma_start(
                wq_sb[:, e0:e0 + ge, :],
                wq_d[e0 * 128:(e0 + ge) * 128, :]
                .rearrange("(f p) c -> p f c", p=128))
            nc.scalar.dma_start(
                wkv_sb[:, e0:e0 + ge, :],
                wkv_d[e0 * 128:(e0 + ge) * 128, :]
                .rearrange("(f p) c -> p f c", p=128))
        bias_sb = const.tile([128, 6], f32, tag="bias")
        cos_sb = const.tile([128, T], bf16, tag="cos")
        sin_sb = const.tile([128, T], bf16, tag="sin")
        tri_sb = const.tile([128, 128], bf16, tag="tri")
        idn_sb = const.tile([128, 128], f32, tag="idn")
        pjt_sb = wpool.tile([128, HPC, E], bf16, tag="pjt")

        def load_consts():
            nc.scalar.dma_start(bias_sb[:], bias_d.rearrange("r p -> p r"))
            nc.scalar.dma_start(cos_sb[:], cos_d[:, :])
            nc.scalar.dma_start(sin_sb[:], sin_d[:, :])
            nc.scalar.dma_start(idn_sb[:], idn_d[:, :])
            nc.scalar.dma_start(tri_sb[:], tri_d[:, :])

        # big SBUF-resident intermediates (bf16): rotated Q, rotated K, V^T
        qrot = big.tile([128, HPC, BT], bf16, tag="qrot")   # [d, h, tok]
        rotK = big.tile([128, BT], bf16, tag="rotK")        # [d, tok]
        vbufT = big.tile([128, BT], bf16, tag="vbuf")       # [tok%128, kc*128+d]

        # PSUM layout (single pool, tags in declared order -> banks):
        #   sc0 [128,1024] banks 0-1 | sc1 [128,1024] banks 2-3
        #   yt0 bank 4 | yt1 bank 5 | po0 bank 6 | po1 bank 7
        sc_tag = ["sc0", "sc1"]
        tp_alt = 0      # V-transpose bank alternator (po0/po1)
        po_alt = 0      # out-proj bank alternator (po0/po1)
        po_ct = 0       # out-proj eviction engine pattern counter

        # ---------------- pass 1: QKV projection + rope ----------------
        # V transposes are deferred into the NEXT chunk's matmul stream so
        # the PE never waits on the ScalarE eviction chain at chunk edges.
        pending_v = []

        def flush_v():
            nonlocal tp_alt
            while pending_v:
                vraw_p, gcol_p = pending_v.pop(0)
                for t4 in range(4):
                    tp = psum.tile([128, 128], f32,
                                   tag=["po0", "po1"][tp_alt],
                                   name=f"tp_{gcol_p}_{t4}")
                    tp_alt ^= 1
                    nc.tensor.transpose(
                        tp[:], vraw_p[:, t4 * 128:(t4 + 1) * 128], idn_sb[:])
                    nc.scalar.copy(vbufT[:, gcol_p + t4 * 128:
                                         gcol_p + (t4 + 1) * 128], tp[:])

        for b in range(B):
            for j in range(CPB):
                gcol = b * T + j * SQ
                tcol = j * SQ
                acc01 = psum.tile([128, 1024], f32, tag="sc0",
                                  name=f"acc01_{b}_{j}")
                acc23 = psum.tile([128, 1024], f32, tag="sc1",
                                  name=f"acc23_{b}_{j}")
                acck = psum.tile([128, SQ], f32, tag="yt0",
                                 name=f"acck_{b}_{j}")
                accv = psum.tile([128, SQ], f32, tag="yt1",
                                 name=f"accv_{b}_{j}")
                accs = [acc01[:, 0:512], acc01[:, 512:1024],
                        acc23[:, 0:512], acc23[:, 512:1024]]
                xplan = ([(0, 1), (1, 1), (2, 2)]
                         + [(e0, GE) for e0 in range(4, ECH, GE)]
                         if (b == 0 and j == 0) else
                         [(e0, GE) for e0 in range(0, ECH, GE)])
                for e0, ge in xplan:
                    xt = xpool.tile([128, ge, SQ], bf16, tag="xt")
                    nc.sync.dma_start(
                        xt[:],
                        xT_d[e0 * 128:(e0 + ge) * 128, gcol:gcol + SQ]
                        .rearrange("(f p) t -> p f t", p=128))
                    for f in range(ge):
                        e = e0 + f
                        st, sp = (e == 0), (e == ECH - 1)
                        for h in range(HPC):
                            nc.tensor.matmul(
                                accs[h], wq_sb[:, e, h * D:(h + 1) * D],
                                xt[:, f, :], start=st, stop=sp)
                        nc.tensor.matmul(acck[:], wkv_sb[:, e, 0:D],
                                         xt[:, f, :], start=st, stop=sp)
                        nc.tensor.matmul(accv[:], wkv_sb[:, e, D:2 * D],
                                         xt[:, f, :], start=st, stop=sp)
                    if b == 0 and j == 0 and e0 >= 4 and e0 < ECH - 4:
                        # just-in-time weight granules: keep early HBM
                        # bandwidth free for the x stream
                        we = e0 + 4
                        nc.scalar.dma_start(
                            wq_sb[:, we:we + GE, :],
                            wq_d[we * 128:(we + GE) * 128, :]
                            .rearrange("(f p) c -> p f c", p=128))
                        nc.scalar.dma_start(
                            wkv_sb[:, we:we + GE, :],
                            wkv_d[we * 128:(we + GE) * 128, :]
                            .rearrange("(f p) c -> p f c", p=128))
                    if e0 == 8:
                        flush_v()
                        if b == 0 and j == 0:
                            load_consts()
                    if b == 1 and j == 2 and e0 == 8:
                        for hq in range(HPC):
                            nc.scalar.dma_start(
                                pjt_sb[:, hq, :],
                                pjt_d[hq * 128:(hq + 1) * 128, :])

                # two-phase rope: evictions split ScalarE (q0,q2,V) /
                # VectorE (q1,q3,K); VEC-evicted heads swap rope pairs via
                # stream_shuffle, ScalarE-evicted ones via strided DMA copies.
                SWAPM = [a + 1 - 2 * (a % 2) for a in range(32)]
                VECH = (1, 3, 4)
                raws, sws = {}, {}
                for c in (0, 1, 2, 3, 4):
                    src = accs[c] if c < 4 else acck[:]
                    raw = spool.tile([128, SQ], bf16, tag="raw", bufs=5,
                                     name=f"raw_{b}_{j}_{c}")
                    if c in VECH:
                        with nc.allow_low_precision(reason="rope evict"):
                            nc.vector.tensor_add(
                                raw[:], src,
                                bias_sb[:, c:c + 1].to_broadcast((128, SQ)))
                    else:
                        nc.scalar.activation(
                            raw[:], src, AF.Identity,
                            bias=bias_sb[:, c:c + 1])
                    raws[c] = raw
                vraw = spool.tile([128, SQ], f32, tag="vraw", bufs=2,
                                  name=f"vraw_{b}_{j}")
                nc.scalar.activation(vraw[:], accv[:], AF.Identity,
                                     bias=bias_sb[:, 5:6])
                pending_v.append((vraw, gcol))
                for c in (0, 1, 2, 3, 4):
                    sw = spool.tile([128, SQ], bf16, tag="sw", bufs=5,
                                    name=f"sw_{b}_{j}_{c}")
                    if c in VECH:
                        nc.vector.stream_shuffle(sw[:], raws[c][:], SWAPM)
                    else:
                        raw3 = raws[c].rearrange("(a two) t -> a two t", two=2)
                        sw3 = sw.rearrange("(a two) t -> a two t", two=2)
                        nc.scalar.dma_start(sw3[:, 1, :], raw3[:, 0, :])
                        nc.scalar.dma_start(sw3[:, 0, :], raw3[:, 1, :])
                    sws[c] = sw
                for c in (0, 1, 2, 3, 4):
                    out_ap = (qrot[:, c, gcol:gcol + SQ] if c < 4
                              else rotK[:, gcol:gcol + SQ])
                    tmp = spool.tile([128, SQ], bf16, tag="rtmp", bufs=3,
                                     name=f"rtmp_{b}_{j}_{c}")
                    nc.vector.tensor_mul(tmp[:], raws[c][:],
                                         cos_sb[:, tcol:tcol + SQ])
                    nc.vector.tensor_mul(sws[c][:], sws[c][:],
                                         sin_sb[:, tcol:tcol + SQ])
                    nc.vector.tensor_add(out_ap, tmp[:], sws[c][:])
        # the last chunk's V transposes are emitted at the start of pass 2

        # ------------- pass 2: attention + output projection -------------
        pend = []   # (kind, block, closure) pending items (PE filler)

        drain = [False]

        def po_group(yts_t, grow, oc, ts8, postg_t, fslot):
            nonlocal po_alt, po_ct
            # post-attention drain: all 8 banks are free -> rotate over 4
            # and split evictions 50/50 so the PE never waits on a bank.
            tags = (["po0", "po1", "yt0", "yt1"] if drain[0]
                    else ["po0", "po1"])
            po_ps = psum.tile([128, SQ], f32, tag=tags[po_alt % len(tags)],
                              name=f"pops_{grow}_{oc}_{ts8}")
            po_alt += 1
            for h in range(HPC):
                nc.tensor.matmul(
                    po_ps[:], yts_t[:, h, ts8 * 128:(ts8 + 1) * 128],
                    pjt_sb[:, h, oc * SQ:(oc + 1) * SQ],
                    start=(h == 0), stop=(h == HPC - 1))
            act_turn = (po_ct % 2 == 1) if drain[0] else (po_ct % 3 == 1)
            if act_turn:
                nc.scalar.copy(postg_t[:, fslot, :], po_ps[:])
            else:
                nc.vector.tensor_copy(postg_t[:, fslot, :], po_ps[:])
            po_ct += 1

        def po_store(postg_t, grow, oc, f0):
            nc.sync.dma_start(
                out_d[grow + f0 * 128:grow + (f0 + 4) * 128,
                      oc * SQ:(oc + 1) * SQ]
                .rearrange("(f p) t -> p f t", p=128),
                postg_t[:])

        def pop_pend(n, reserve=0):
            for _ in range(n):
                if len(pend) <= reserve:
                    return
                pend.pop(0)[2]()

        def pop_stale_norms(cur_blk):
            # emit any previous block's norm_muls (their sums chains are long
            # done) before this block's first write to the bufs=1 yty slot
            while any(k == "norm" and bl < cur_blk for k, bl, _ in pend):
                pend.pop(0)[2]()

        # the last pass-1 chunk's V transposes become the first PE filler
        # items (pass-2 j=0 has no out-proj work yet); evictions alternate
        # VectorE/ScalarE so the exp stream isn't delayed on ACT.
        def v_tr_one(vraw_p, gcol_p, t4, on_vec):
            nonlocal tp_alt
            tp = psum.tile([128, 128], f32, tag=["po0", "po1"][tp_alt],
                           name=f"tpf_{gcol_p}_{t4}")
            tp_alt ^= 1
            nc.tensor.transpose(
                tp[:], vraw_p[:, t4 * 128:(t4 + 1) * 128], idn_sb[:])
            dst = vbufT[:, gcol_p + t4 * 128:gcol_p + (t4 + 1) * 128]
            if on_vec:
                nc.vector.tensor_copy(dst, tp[:])
            else:
                nc.scalar.copy(dst, tp[:])
        while pending_v:
            vraw_p, gcol_p = pending_v.pop(0)
            for t4 in range(4):
                pend.append(("po", -1,
                             lambda v=vraw_p, g=gcol_p, t=t4, ov=(t4 % 2 == 0):
                             v_tr_one(v, g, t, ov)))

        def norm_mul(yts_t, yty_t, rbb_t, jj_, h):
            sidx = jj_ * 4 + h
            nc.vector.tensor_mul(
                yts_t[:, h, jj_ * SQ:(jj_ + 1) * SQ],
                yty_t[:, sidx, :], rbb_t[:])

        blk_ct = [0]

        for b in range(B):
            # process j-groups densest-first (j=3..0): the first group runs
            # with an empty filler backlog, so give it the best PE/exp ratio;
            # j=0 (mostly-masked, thin PE) runs last against a deep backlog.
            for hh in (1, 0):
                blk = blk_ct[0]
                blk_ct[0] += 1
                need_norm_drain = [True]
                yts = ypool.tile([128, HPC, HB], bf16, tag="yts",
                                 name=f"yts_{b}_{hh}")
                # reuse the (pass-1-only) wkv weight slot for the
                # unnormalized attention staging buffer: same pool tag ->
                # same SBUF bytes, WAR deps order it after the last QKV read.
                yty = wpool.tile([128, 8, SQ], bf16, tag="wkv", bufs=1,
                                 name=f"yty_{b}_{hh}")
                for jj in (1, 0):
                    j = hh * 2 + jj
                    last_jj = (b == 1 and hh == 0 and jj == 0)
                    gcol = b * T + j * SQ
                    nkc = 4 * j + 4
                    for pr in range(2):          # head pairs (2h per pass)
                        hs = (2 * pr, 2 * pr + 1)
                        yt_ps = [psum.tile([128, SQ], f32,
                                           tag=["yt0", "yt1"][i],
                                           name=f"yt_{b}_{j}_{h}")
                                 for i, h in enumerate(hs)]
                        acc2 = accp.tile([128, 1024], bf16, tag="acc",
                                         name=f"acc2_{b}_{j}_{pr}")
                        prev = None

                        def emit_av(kc, es_t, off):
                            st, sp = (kc == 0), (kc == nkc - 1)
                            koff = b * T + kc * 128
                            for i in range(2):
                                nc.tensor.matmul(
                                    yt_ps[i][:, off:SQ],
                                    vbufT[:, koff:koff + 128],
                                    es_t[:, i * SQ + off:(i + 1) * SQ],
                                    start=st, stop=sp)

                        for kc in range(nkc):
                            koff = b * T + kc * 128
                            off = 128 * max(0, kc - 4 * j)
                            s2 = psum.tile([128, 1024], f32,
                                           tag=sc_tag[kc % 2],
                                           name=f"s_{b}_{j}_{kc}_{pr}")
                            for i, h in enumerate(hs):
                                nc.tensor.matmul(
                                    s2[:, i * SQ + off:(i + 1) * SQ],
                                    rotK[:, koff:koff + 128],
                                    qrot[:, h, gcol + off:gcol + SQ],
                                    start=True, stop=True)
                            es = espool.tile([128, 1024], bf16, tag="es",
                                             name=f"es_{b}_{j}_{kc}_{pr}")
                            nc.scalar.activation(es[:], s2[:],
                                                 AF.Exp, scale=INV_SQRT_D)
                            if off > 0 or kc == 4 * j:
                                # mask the leading 128-wide causal triangle
                                # (DVE, NOT gpsimd: mixing ops on gpsimd
                                # thrashes its loadable Q7 library against
                                # partition_broadcast, ~6us per swap)
                                for i in range(2):
                                    nc.vector.tensor_mul(
                                        es[:, i * SQ + off:i * SQ + off + 128],
                                        es[:, i * SQ + off:i * SQ + off + 128],
                                        tri_sb[:])
                            # softmax denominators: accumulate exp on DVE
                            if kc == 0:
                                nc.vector.tensor_copy(acc2[:], es[:])
                            elif off == 0:
                                nc.vector.tensor_add(acc2[:], acc2[:], es[:])
                            else:
                                for i in range(2):
                                    nc.vector.tensor_add(
                                        acc2[:, i * SQ + off:(i + 1) * SQ],
                                        acc2[:, i * SQ + off:(i + 1) * SQ],
                                        es[:, i * SQ + off:(i + 1) * SQ])
                            if prev is not None:
                                emit_av(kc - 1, *prev)
                            prev = (es, off)
                            pop_pend(2, reserve=14)
                        emit_av(nkc - 1, *prev)

                        if need_norm_drain[0]:
                            pop_stale_norms(blk)
                            need_norm_drain[0] = False
                        # stage attention outputs out of PSUM right away
                        for i, h in enumerate(hs):
                            sidx = jj * 4 + pr * 2 + i
                            nc.vector.tensor_copy(yty[:, sidx, :],
                                                  yt_ps[i][:])
                        pop_pend(2)
                        # per-pair softmax denominators: GPSIMD all-reduce
                        # over the 128 key-residues leaves the sums replicated
                        # on every partition -- exactly the layout the
                        # normalization multiply needs (no matmul, no
                        # broadcast DMAs; GPSIMD runs only this one Q7 op).
                        for i in range(2):
                            sbc = spool.tile([128, SQ], f32, tag="vraw",
                                             bufs=2,
                                             name=f"sbc_{b}_{j}_{pr}_{i}")
                            nc.gpsimd.partition_all_reduce(
                                sbc[:], acc2[:, i * SQ:(i + 1) * SQ],
                                channels=128,
                                reduce_op=bass_isa.ReduceOp.add)
                            rinv = spool.tile([128, SQ], f32, tag="vraw",
                                              bufs=2,
                                              name=f"rinv_{b}_{j}_{pr}_{i}")
                            nc.vector.reciprocal_approx_fast(rinv[:], sbc[:])
                            rbb = spool.tile([128, SQ], bf16, tag="rb",
                                             bufs=4,
                                             name=f"rbb_{b}_{j}_{pr}_{i}")
                            with nc.allow_low_precision(reason="recip cast"):
                                nc.vector.tensor_copy(rbb[:], rinv[:])
                            pend.append(("norm", blk,
                                         lambda y=yts, yy=yty, dd=rbb,
                                         a=jj, q=pr * 2 + i:
                                         norm_mul(y, yy, dd, a, q)))

                    grow = b * T + hh * HB
                    for oc in range(8):
                        postg = ppool.tile([128, 4, SQ], f16, tag="po",
                                           name=f"postg_{b}_{hh}_{jj}_{oc}")
                        for k4, ts8 in enumerate(range(jj * 4, jj * 4 + 4)):
                            pend.append(("po", blk,
                                         lambda y=yts, g=grow, o=oc, t=ts8,
                                         pt=postg, k=k4:
                                         po_group(y, g, o, t, pt, k)))
                        pend.append(("po", blk,
                                     lambda pt=postg, g=grow, o=oc, f0=jj * 4:
                                     po_store(pt, g, o, f0)))
        drain[0] = True
        pop_pend(len(pend))

    nc.compile()
    return nc


_PROG = None


def kernel(x, freq_cos, freq_sin, w_q_w, w_q_b, w_kv_w, w_kv_b, proj_w, proj_b,
           start_pos=0, **_unused):
    global _PROG
    import ml_dtypes
    from concourse.bass_utils import run_bass_kernel_spmd

    bf16 = ml_dtypes.bfloat16

    x = np.asarray(x, np.float32)
    freq_cos = np.asarray(freq_cos, np.float32)
    freq_sin = np.asarray(freq_sin, np.float32)
    w_q_w = np.asarray(w_q_w, np.float32)
    w_q_b = np.asarray(w_q_b, np.float32)
    w_kv_w = np.asarray(w_kv_w, np.float32)
    w_kv_b = np.asarray(w_kv_b, np.float32)
    proj_w = np.asarray(proj_w, np.float32)
    proj_b = np.asarray(proj_b, np.float32)

    xT = np.ascontiguousarray(x.reshape(BT, E).T).astype(bf16)

    cosE = np.repeat(freq_cos.T, 2, axis=0).astype(np.float32)        # [128, T]
    sinE = np.repeat(freq_sin.T, 2, axis=0).astype(np.float32)
    sinS = sinE.copy()
    sinS[0::2, :] *= -1.0                                             # even rows -sin
    cosE = cosE.astype(bf16)
    sinS = sinS.astype(bf16)

    kp = np.arange(128)[:, None]
    qq = np.arange(128)[None, :]
    triM = (qq >= kp).astype(bf16)                                    # [128, 128]

    ident = np.eye(128, dtype=np.float32)

    if _PROG is None:
        _PROG = _build_program()

    in_maps = []
    for c in range(NCORES):
        wq_c = np.ascontiguousarray(
            w_q_w[c * 512:(c + 1) * 512, :].T).astype(bf16)            # [E, 512]
        kT = w_kv_w[c * D:(c + 1) * D, :].T                            # [E, 128]
        vT = w_kv_w[8 * D + c * D:8 * D + (c + 1) * D, :].T
        wkv_c = np.ascontiguousarray(
            np.concatenate([kT, vT], axis=1)).astype(bf16)             # [E, 256]
        biases = np.zeros((6, 128), np.float32)
        biases[0:4, :] = w_q_b[c * 512:(c + 1) * 512].reshape(4, 128)
        biases[4, :] = w_kv_b[c * D:(c + 1) * D]
        biases[5, :] = w_kv_b[8 * D + c * D:8 * D + (c + 1) * D]
        pjt_c = np.ascontiguousarray(
            proj_w[:, c * 512:(c + 1) * 512].T).astype(bf16)           # [512, E]
        in_maps.append({
            "xT": xT, "wqT": wq_c, "wkvT": wkv_c, "biases": biases,
            "cosE": cosE, "sinS": sinS, "triM": triM,
            "projT": pjt_c, "ident": ident,
        })

    res = run_bass_kernel_spmd(_PROG, in_maps, core_ids=list(range(NCORES)))
    out = np.zeros((BT, E), np.float32)
    for c in range(NCORES):
        out += res.results[c]["yp"].astype(np.float32)
    out = out + proj_b[None, :].astype(np.float32)
    return out.reshape(B, T, E).astype(np.float32)


# revision 17
# speedup vs baseline: 1.0786x; 1.0786x over previous
"""Llama GQA attention block, tensor-parallel over heads across 8 TRN2 NeuronCores.

Contract: kernel(**inputs) takes the FULL inputs of the reference
(x, freq_cos, freq_sin, w_q_w, w_q_b, w_kv_w, w_kv_b, proj_w, proj_b, start_pos)
and returns the FULL output (B, T, N_EMBD) float32.

Sharding: core c owns query heads 4c..4c+3 and KV head c, plus proj rows
c*512..(c+1)*512. Each core computes a partial projection output (fp16); the
host sums the 8 partials and adds proj_b.

v2 design (vs the 920us baseline):
- softmax row sums no longer use a per-kc ones-matmul on the PE (was 164K
  cycles/core).  exp tiles accumulate into a bf16 SBUF accumulator on the DVE
  (2x packed mode); a single 4-column selector matmul per (b,j) then reduces
  the 128 key-residues, yielding a [4,512] sums tile (one bank, all 4 heads).
- causal trimming: in diagonal 512-blocks the score/AV matmuls and the DVE
  accumulation only cover the valid query range (moving-dim slice); only the
  leading 128-wide triangle needs a mask multiply (GPSIMD, [128,128]).
- both heads of a pair share one 2-bank PSUM score tile [128,1024]; exp runs
  as a single ACTIVATE over 1024 columns (amortizes the ~293ns ACT overhead).
- x is loaded in 4-e-chunk granules (one 512KB DMA for [128,4,512]) and
  output tiles are stored 4-at-a-time (512KB DMAs), cutting HWDGE queue
  dispatch time ~4x and removing the pass-1 chunk-boundary DMA bubbles.
- out-projection stays a PE filler stream popped during attention (exp on the
  ACT engine is the per-kc critical path; filler keeps the PE saturated).
"""

import math
import numpy as np
from contextlib import ExitStack

# Problem constants (hardcoded per the harness contract).
B = 2
T = 2048
E = 4096
D = 128          # head dim
NCORES = 8
HPC = 4          # query heads per core
BT = B * T       # 4096
SQ = 512         # token chunk (matmul moving dim)
ECH = E // 128   # 32 contraction chunks
GE = 4           # e-chunks per x/weight DMA granule
NG = ECH // GE   # granules per token chunk
CPB = T // SQ    # 4 tok chunks per batch
INV_SQRT_D = 1.0 / math.sqrt(D)
HB = 1024        # half-batch token span for the output projection stage


def _build_program():
    import concourse.bass as bass  # noqa: F401
    import concourse.bass_isa as bass_isa
    import concourse.mybir as mybir
    import concourse.tile as tile
    from concourse import bacc

    f32 = mybir.dt.float32
    bf16 = mybir.dt.bfloat16
    f16 = mybir.dt.float16
    AF = mybir.ActivationFunctionType

    nc = bacc.Bacc("TRN2", target_bir_lowering=False, debug=False)

    xT_d = nc.dram_tensor("xT", [E, BT], bf16, kind="ExternalInput")
    wq_d = nc.dram_tensor("wqT", [E, HPC * D], bf16, kind="ExternalInput")
    wkv_d = nc.dram_tensor("wkvT", [E, 2 * D], bf16, kind="ExternalInput")
    bias_d = nc.dram_tensor("biases", [6, 128], f32, kind="ExternalInput")
    cos_d = nc.dram_tensor("cosE", [128, T], bf16, kind="ExternalInput")
    sin_d = nc.dram_tensor("sinS", [128, T], bf16, kind="ExternalInput")
    tri_d = nc.dram_tensor("triM", [128, 128], bf16, kind="ExternalInput")
    sel_d = nc.dram_tensor("sel16", [128, 16], bf16, kind="ExternalInput")
    pjt_d = nc.dram_tensor("projT", [HPC * D, E], bf16, kind="ExternalInput")
    idn_d = nc.dram_tensor("ident", [128, 128], f32, kind="ExternalInput")
    out_d = nc.dram_tensor("yp", [BT, E], f16, kind="ExternalOutput")

    with tile.TileContext(nc) as tc, ExitStack() as ctx:
        const = ctx.enter_context(tc.tile_pool(name="const", bufs=1))
        wpool = ctx.enter_context(tc.tile_pool(name="wpool", bufs=1))
        big = ctx.enter_context(tc.tile_pool(name="big", bufs=1))
        xpool = ctx.enter_context(tc.tile_pool(name="xpool", bufs=3))
        espool = ctx.enter_context(tc.tile_pool(name="espool", bufs=3))
        accp = ctx.enter_context(tc.tile_pool(name="accp", bufs=2))
        spool = ctx.enter_context(tc.tile_pool(name="spool", bufs=2))
        ppool = ctx.enter_context(tc.tile_pool(name="ppool", bufs=2))
        ypool = ctx.enter_context(tc.tile_pool(name="ypool", bufs=2))
        psum = ctx.enter_context(tc.tile_pool(name="ps", bufs=1, space="PSUM"))

        # ---- weights / constants resident in SBUF ----
        wq_sb = wpool.tile([128, ECH, HPC * D], bf16, tag="wq")
        wkv_sb = wpool.tile([128, ECH, 2 * D], bf16, tag="wkv")
        # granule loads on the scalar HWDGE ring (x rides the sync ring).
        # wq/wkv granules interleave so the first k/v matmuls aren't stuck
        # behind all 4MB of wq; the leading granules are small for fast start.
        WGRAN = [(0, 1), (1, 1), (2, 2), (4, 4)]
        for e0, ge in WGRAN:
            nc.scalar.dma_start(
                wq_sb[:, e0:e0 + ge, :],
                wq_d[e0 * 128:(e0 + ge) * 128, :]
                .rearrange("(f p) c -> p f c", p=128))
            nc.scalar.dma_start(
                wkv_sb[:, e0:e0 + ge, :],
                wkv_d[e0 * 128:(e0 + ge) * 128, :]
                .rearrange("(f p) c -> p f c", p=128))
        bias_sb = const.tile([128, 6], f32, tag="bias")
        cos_sb = const.tile([128, T], bf16, tag="cos")
        sin_sb = const.tile([128, T], bf16, tag="sin")
        tri_sb = const.tile([128, 128], bf16, tag="tri")
        sel_sb = const.tile([128, 16], bf16, tag="sel")
        idn_sb = const.tile([128, 128], f32, tag="idn")
        pjt_sb = wpool.tile([128, HPC, E], bf16, tag="pjt")

        def load_consts():
            nc.scalar.dma_start(bias_sb[:], bias_d.rearrange("r p -> p r"))
            nc.scalar.dma_start(cos_sb[:], cos_d[:, :])
            nc.scalar.dma_start(sin_sb[:], sin_d[:, :])
            nc.scalar.dma_start(idn_sb[:], idn_d[:, :])
            nc.scalar.dma_start(sel_sb[:], sel_d[:, :])
            nc.scalar.dma_start(tri_sb[:], tri_d[:, :])

        # big SBUF-resident intermediates (bf16): rotated Q, rotated K, V^T
        qrot = big.tile([128, HPC, BT], bf16, tag="qrot")   # [d, h, tok]
        rotK = big.tile([128, BT], bf16, tag="rotK")        # [d, tok]
        vbufT = big.tile([128, BT], bf16, tag="vbuf")       # [tok%128, kc*128+d]

        # PSUM layout (single pool, tags in declared order -> banks):
        #   sc0 [128,1024] banks 0-1 | sc1 [128,1024] banks 2-3
        #   yt0 bank 4 | yt1 bank 5 | po0 bank 6 | po1 bank 7
        sc_tag = ["sc0", "sc1"]
        tp_alt = 0      # V-transpose bank alternator (po0/po1)
        po_alt = 0      # out-proj bank alternator (po0/po1)
        po_ct = 0       # out-proj eviction engine pattern counter

        # ---------------- pass 1: QKV projection + rope ----------------
        # V transposes are deferred into the NEXT chunk's matmul stream so
        # the PE never waits on the ScalarE eviction chain at chunk edges.
        pending_v = []

        def flush_v():
            nonlocal tp_alt
            while pending_v:
                vraw_p, gcol_p = pending_v.pop(0)
                for t4 in range(4):
                    tp = psum.tile([128, 128], f32,
                                   tag=["po0", "po1"][tp_alt],
                                   name=f"tp_{gcol_p}_{t4}")
                    tp_alt ^= 1
                    nc.tensor.transpose(
                        tp[:], vraw_p[:, t4 * 128:(t4 + 1) * 128], idn_sb[:])
                    nc.scalar.copy(vbufT[:, gcol_p + t4 * 128:
                                         gcol_p + (t4 + 1) * 128], tp[:])

        for b in range(B):
            for j in range(CPB):
                gcol = b * T + j * SQ
                tcol = j * SQ
                acc01 = psum.tile([128, 1024], f32, tag="sc0",
                                  name=f"acc01_{b}_{j}")
                acc23 = psum.tile([128, 1024], f32, tag="sc1",
                                  name=f"acc23_{b}_{j}")
                acck = psum.tile([128, SQ], f32, tag="yt0",
                                 name=f"acck_{b}_{j}")
                accv = psum.tile([128, SQ], f32, tag="yt1",
                                 name=f"accv_{b}_{j}")
                accs = [acc01[:, 0:512], acc01[:, 512:1024],
                        acc23[:, 0:512], acc23[:, 512:1024]]
                xplan = ([(0, 1), (1, 1), (2, 2)]
                         + [(e0, GE) for e0 in range(4, ECH, GE)]
                         if (b == 0 and j == 0) else
                         [(e0, GE) for e0 in range(0, ECH, GE)])
                for e0, ge in xplan:
                    xt = xpool.tile([128, ge, SQ], bf16, tag="xt")
                    nc.sync.dma_start(
                        xt[:],
                        xT_d[e0 * 128:(e0 + ge) * 128, gcol:gcol + SQ]
                        .rearrange("(f p) t -> p f t", p=128))
                    for f in range(ge):
                        e = e0 + f
                        st, sp = (e == 0), (e == ECH - 1)
                        for h in range(HPC):
                            nc.tensor.matmul(
                                accs[h], wq_sb[:, e, h * D:(h + 1) * D],
                                xt[:, f, :], start=st, stop=sp)
                        nc.tensor.matmul(acck[:], wkv_sb[:, e, 0:D],
                                         xt[:, f, :], start=st, stop=sp)
                        nc.tensor.matmul(accv[:], wkv_sb[:, e, D:2 * D],
                                         xt[:, f, :], start=st, stop=sp)
                    if b == 0 and j == 0 and e0 >= 4 and e0 < ECH - 4:
                        # just-in-time weight granules: keep early HBM
                        # bandwidth free for the x stream
                        we = e0 + 4
                        nc.scalar.dma_start(
                            wq_sb[:, we:we + GE, :],
                            wq_d[we * 128:(we + GE) * 128, :]
                            .rearrange("(f p) c -> p f c", p=128))
                        nc.scalar.dma_start(
                            wkv_sb[:, we:we + GE, :],
                            wkv_d[we * 128:(we + GE) * 128, :]
                            .rearrange("(f p) c -> p f c", p=128))
                    if e0 == 8:
                        flush_v()
                        if b == 0 and j == 0:
                            load_consts()
                    if b == 1 and j == 2 and e0 == 8:
                        for hq in range(HPC):
                            nc.scalar.dma_start(
                                pjt_sb[:, hq, :],
                                pjt_d[hq * 128:(hq + 1) * 128, :])

                # two-phase rope: evictions split ScalarE (q0,q2,V) /
                # VectorE (q1,q3,K); VEC-evicted heads swap rope pairs via
                # stream_shuffle, ScalarE-evicted ones via strided DMA copies.
                SWAPM = [a + 1 - 2 * (a % 2) for a in range(32)]
                VECH = (1, 3, 4)
                raws, sws = {}, {}
                for c in (0, 1, 2, 3, 4):
                    src = accs[c] if c < 4 else acck[:]
                    raw = spool.tile([128, SQ], bf16, tag="raw", bufs=5,
                                     name=f"raw_{b}_{j}_{c}")
                    if c in VECH:
                        with nc.allow_low_precision(reason="rope evict"):
                            nc.vector.tensor_add(
                                raw[:], src,
                                bias_sb[:, c:c + 1].to_broadcast((128, SQ)))
                    else:
                        nc.scalar.activation(
                            raw[:], src, AF.Identity,
                            bias=bias_sb[:, c:c + 1])
                    raws[c] = raw
                vraw = spool.tile([128, SQ], f32, tag="vraw", bufs=2,
                                  name=f"vraw_{b}_{j}")
                nc.scalar.activation(vraw[:], accv[:], AF.Identity,
                                     bias=bias_sb[:, 5:6])
                pending_v.append((vraw, gcol))
                for c in (0, 1, 2, 3, 4):
                    sw = spool.tile([128, SQ], bf16, tag="sw", bufs=5,
                                    name=f"sw_{b}_{j}_{c}")
                    if c in VECH:
                        nc.vector.stream_shuffle(sw[:], raws[c][:], SWAPM)
                    else:
                        raw3 = raws[c].rearrange("(a two) t -> a two t", two=2)
                        sw3 = sw.rearrange("(a two) t -> a two t", two=2)
                        nc.scalar.dma_start(sw3[:, 1, :], raw3[:, 0, :])
                        nc.scalar.dma_start(sw3[:, 0, :], raw3[:, 1, :])
                    sws[c] = sw
                for c in (0, 1, 2, 3, 4):
                    out_ap = (qrot[:, c, gcol:gcol + SQ] if c < 4
                              else rotK[:, gcol:gcol + SQ])
                    tmp = spool.tile([128, SQ], bf16, tag="rtmp", bufs=3,
                                     name=f"rtmp_{b}_{j}_{c}")
                    nc.vector.tensor_mul(tmp[:], raws[c][:],
                                         cos_sb[:, tcol:tcol + SQ])
                    nc.vector.tensor_mul(sws[c][:], sws[c][:],
                                         sin_sb[:, tcol:tcol + SQ])
                    nc.vector.tensor_add(out_ap, tmp[:], sws[c][:])
        # the last chunk's V transposes are emitted at the start of pass 2

        # ------------- pass 2: attention + output projection -------------
        pend = []   # (kind, block, closure) pending items (PE filler)

        drain = [False]

        def po_group(yts_t, grow, oc, ts8, postg_t, fslot):
            nonlocal po_alt, po_ct
            # post-attention drain: all 8 banks are free -> rotate over 4
            # and split evictions 50/50 so the PE never waits on a bank.
            tags = (["po0", "po1", "yt0", "yt1"] if drain[0]
                    else ["po0", "po1"])
            po_ps = psum.tile([128, SQ], f32, tag=tags[po_alt % len(tags)],
                              name=f"pops_{grow}_{oc}_{ts8}")
            po_alt += 1
            for h in range(HPC):
                nc.tensor.matmul(
                    po_ps[:], yts_t[:, h, ts8 * 128:(ts8 + 1) * 128],
                    pjt_sb[:, h, oc * SQ:(oc + 1) * SQ],
                    start=(h == 0), stop=(h == HPC - 1))
            act_turn = (po_ct % 2 == 1) if drain[0] else (po_ct % 3 == 1)
            if act_turn:
                nc.scalar.copy(postg_t[:, fslot, :], po_ps[:])
            else:
                nc.vector.tensor_copy(postg_t[:, fslot, :], po_ps[:])
            po_ct += 1

        def po_store(postg_t, grow, oc, f0):
            nc.sync.dma_start(
                out_d[grow + f0 * 128:grow + (f0 + 4) * 128,
                      oc * SQ:(oc + 1) * SQ]
                .rearrange("(f p) t -> p f t", p=128),
                postg_t[:])

        def pop_pend(n, reserve=0):
            for _ in range(n):
                if len(pend) <= reserve:
                    return
                pend.pop(0)[2]()

        def pop_stale_norms(cur_blk):
            # emit any previous block's norm_muls (their sums chains are long
            # done) before this block's first write to the bufs=1 yty slot
            while any(k == "norm" and bl < cur_blk for k, bl, _ in pend):
                pend.pop(0)[2]()

        # the last pass-1 chunk's V transposes become the first PE filler
        # items (pass-2 j=0 has no out-proj work yet); evictions alternate
        # VectorE/ScalarE so the exp stream isn't delayed on ACT.
        def v_tr_one(vraw_p, gcol_p, t4, on_vec):
            nonlocal tp_alt
            tp = psum.tile([128, 128], f32, tag=["po0", "po1"][tp_alt],
                           name=f"tpf_{gcol_p}_{t4}")
            tp_alt ^= 1
            nc.tensor.transpose(
                tp[:], vraw_p[:, t4 * 128:(t4 + 1) * 128], idn_sb[:])
            dst = vbufT[:, gcol_p + t4 * 128:gcol_p + (t4 + 1) * 128]
            if on_vec:
                nc.vector.tensor_copy(dst, tp[:])
            else:
                nc.scalar.copy(dst, tp[:])
        while pending_v:
            vraw_p, gcol_p = pending_v.pop(0)
            for t4 in range(4):
                pend.append(("po", -1,
                             lambda v=vraw_p, g=gcol_p, t=t4, ov=(t4 % 2 == 0):
                             v_tr_one(v, g, t, ov)))

        def norm_mul(yts_t, yty_t, rr0_t, jj_, pr_, i_):
            h = pr_ * 2 + i_
            sidx = jj_ * 4 + h
            rb = spool.tile([128, SQ], bf16, tag="rb", bufs=2,
                            name=f"rb_{sidx}")
            nc.gpsimd.partition_broadcast(
                rb[:], rr0_t[0:1, i_, :], channels=128)
            nc.vector.tensor_mul(
                yts_t[:, h, jj_ * SQ:(jj_ + 1) * SQ],
                yty_t[:, sidx, :], rb[:])

        blk_ct = [0]

        for b in range(B):
            # process j-groups densest-first (j=3..0): the first group runs
            # with an empty filler backlog, so give it the best PE/exp ratio;
            # j=0 (mostly-masked, thin PE) runs last against a deep backlog.
            for hh in (1, 0):
                blk = blk_ct[0]
                blk_ct[0] += 1
                need_norm_drain = [True]
                yts = ypool.tile([128, HPC, HB], bf16, tag="yts",
                                 name=f"yts_{b}_{hh}")
                # reuse the (pass-1-only) wkv weight slot for the
                # unnormalized attention staging buffer: same pool tag ->
                # same SBUF bytes, WAR deps order it after the last QKV read.
                yty = wpool.tile([128, 8, SQ], bf16, tag="wkv", bufs=1,
                                 name=f"yty_{b}_{hh}")
                for jj in (1, 0):
                    j = hh * 2 + jj
                    last_jj = (b == 1 and hh == 0 and jj == 0)
                    gcol = b * T + j * SQ
                    nkc = 4 * j + 4
                    for pr in range(2):          # head pairs (2h per pass)
                        hs = (2 * pr, 2 * pr + 1)
                        yt_ps = [psum.tile([128, SQ], f32,
                                           tag=["yt0", "yt1"][i],
                                           name=f"yt_{b}_{j}_{h}")
                                 for i, h in enumerate(hs)]
                        acc2 = accp.tile([128, 1024], bf16, tag="acc",
                                         name=f"acc2_{b}_{j}_{pr}")
                        prev = None

                        def emit_av(kc, es_t, off):
                            st, sp = (kc == 0), (kc == nkc - 1)
                            koff = b * T + kc * 128
                            for i in range(2):
                                nc.tensor.matmul(
                                    yt_ps[i][:, off:SQ],
                                    vbufT[:, koff:koff + 128],
                                    es_t[:, i * SQ + off:(i + 1) * SQ],
                                    start=st, stop=sp)

                        for kc in range(nkc):
                            koff = b * T + kc * 128
                            off = 128 * max(0, kc - 4 * j)
                            s2 = psum.tile([128, 1024], f32,
                                           tag=sc_tag[kc % 2],
                                           name=f"s_{b}_{j}_{kc}_{pr}")
                            for i, h in enumerate(hs):
                                nc.tensor.matmul(
                                    s2[:, i * SQ + off:(i + 1) * SQ],
                                    rotK[:, koff:koff + 128],
                                    qrot[:, h, gcol + off:gcol + SQ],
                                    start=True, stop=True)
                            es = espool.tile([128, 1024], bf16, tag="es",
                                             name=f"es_{b}_{j}_{kc}_{pr}")
                            nc.scalar.activation(es[:], s2[:],
                                                 AF.Exp, scale=INV_SQRT_D)
                            if off > 0 or kc == 4 * j:
                                # mask the leading 128-wide causal triangle
                                # (DVE, NOT gpsimd: mixing ops on gpsimd
                                # thrashes its loadable Q7 library against
                                # partition_broadcast, ~6us per swap)
                                for i in range(2):
                                    nc.vector.tensor_mul(
                                        es[:, i * SQ + off:i * SQ + off + 128],
                                        es[:, i * SQ + off:i * SQ + off + 128],
                                        tri_sb[:])
                            # softmax denominators: accumulate exp on DVE
                            if kc == 0:
                                nc.vector.tensor_copy(acc2[:], es[:])
                            elif off == 0:
                                nc.vector.tensor_add(acc2[:], acc2[:], es[:])
                            else:
                                for i in range(2):
                                    nc.vector.tensor_add(
                                        acc2[:, i * SQ + off:(i + 1) * SQ],
                                        acc2[:, i * SQ + off:(i + 1) * SQ],
                                        es[:, i * SQ + off:(i + 1) * SQ])
                            if prev is not None:
                                emit_av(kc - 1, *prev)
                            prev = (es, off)
                            pop_pend(2, reserve=14)
                        emit_av(nkc - 1, *prev)

                        if need_norm_drain[0]:
                            pop_stale_norms(blk)
                            need_norm_drain[0] = False
                        # stage attention outputs out of PSUM right away
                        for i, h in enumerate(hs):
                            sidx = jj * 4 + pr * 2 + i
                            nc.vector.tensor_copy(yty[:, sidx, :],
                                                  yt_ps[i][:])
                        pop_pend(2)
                        # per-pair softmax sums: 2 selector matmuls reduce the
                        # bf16 accumulator's 128 key-residues; the whole
                        # reciprocal chain stays on DVE (no ACT-queue hops)
                        sums2 = psum.tile([2, SQ], f32, tag="yt0",
                                          name=f"sums2_{b}_{j}_{pr}")
                        for i in range(2):
                            nc.tensor.matmul(
                                sums2[:], sel_sb[:, 4 * i:4 * i + 2],
                                acc2[:, i * SQ:(i + 1) * SQ],
                                start=(i == 0), stop=(i == 1))
                        stg = spool.tile([2, SQ], f32, tag="vraw", bufs=2,
                                         name=f"stg_{b}_{j}_{pr}")
                        nc.vector.tensor_copy(stg[:], sums2[:])
                        rrf = spool.tile([2, SQ], f32, tag="vraw", bufs=2,
                                         name=f"rrf_{b}_{j}_{pr}")
                        nc.vector.reciprocal_approx_fast(rrf[:], stg[:])
                        rrb = spool.tile([2, SQ], bf16, tag="rrb", bufs=2,
                                         name=f"rrb_{b}_{j}_{pr}")
                        with nc.allow_low_precision(reason="softmax recip"):
                            nc.vector.tensor_copy(rrb[:], rrf[:])
                        # partition_broadcast only reads partition 0: hop the
                        # rows there via tiny SBUF->SBUF DMAs (idle sync ring)
                        rr0 = spool.tile([1, 2, SQ], bf16, tag="rr0", bufs=2,
                                         name=f"rr0_{b}_{j}_{pr}")
                        for i in range(2):
                            nc.sync.dma_start(rr0[0:1, i, :], rrb[i:i + 1, :])
                        for i in range(2):
                            pend.append(("norm", blk,
                                         lambda y=yts, yy=yty, dd=rr0,
                                         a=jj, p=pr, q=i:
                                         norm_mul(y, yy, dd, a, p, q)))

                    grow = b * T + hh * HB
                    for oc in range(8):
                        postg = ppool.tile([128, 4, SQ], f16, tag="po",
                                           name=f"postg_{b}_{hh}_{jj}_{oc}")
                        for k4, ts8 in enumerate(range(jj * 4, jj * 4 + 4)):
                            pend.append(("po", blk,
                                         lambda y=yts, g=grow, o=oc, t=ts8,
                                         pt=postg, k=k4:
                                         po_group(y, g, o, t, pt, k)))
                        pend.append(("po", blk,
                                     lambda pt=postg, g=grow, o=oc, f0=jj * 4:
                                     po_store(pt, g, o, f0)))
        drain[0] = True
        pop_pend(len(pend))

    nc.compile()
    return nc


_PROG = None


def kernel(x, freq_cos, freq_sin, w_q_w, w_q_b, w_kv_w, w_kv_b, proj_w, proj_b,
           start_pos=0, **_unused):
    global _PROG
    import ml_dtypes
    from concourse.bass_utils import run_bass_kernel_spmd

    bf16 = ml_dtypes.bfloat16

    x = np.asarray(x, np.float32)
    freq_cos = np.asarray(freq_cos, np.float32)
    freq_sin = np.asarray(freq_sin, np.float32)
    w_q_w = np.asarray(w_q_w, np.float32)
    w_q_b = np.asarray(w_q_b, np.float32)
    w_kv_w = np.asarray(w_kv_w, np.float32)
    w_kv_b = np.asarray(w_kv_b, np.float32)
    proj_w = np.asarray(proj_w, np.float32)
    proj_b = np.asarray(proj_b, np.float32)

    xT = np.ascontiguousarray(x.reshape(BT, E).T).astype(bf16)

    cosE = np.repeat(freq_cos.T, 2, axis=0).astype(np.float32)        # [128, T]
    sinE = np.repeat(freq_sin.T, 2, axis=0).astype(np.float32)
    sinS = sinE.copy()
    sinS[0::2, :] *= -1.0                                             # even rows -sin
    cosE = cosE.astype(bf16)
    sinS = sinS.astype(bf16)

    kp = np.arange(128)[:, None]
    qq = np.arange(128)[None, :]
    triM = (qq >= kp).astype(bf16)                                    # [128, 128]

    sel16 = np.zeros((128, 16), np.float32)
    for h in range(4):
        sel16[:, 4 * h + h] = 1.0
    sel16 = sel16.astype(bf16)

    ident = np.eye(128, dtype=np.float32)

    if _PROG is None:
        _PROG = _build_program()

    in_maps = []
    for c in range(NCORES):
        wq_c = np.ascontiguousarray(
            w_q_w[c * 512:(c + 1) * 512, :].T).astype(bf16)            # [E, 512]
        kT = w_kv_w[c * D:(c + 1) * D, :].T                            # [E, 128]
        vT = w_kv_w[8 * D + c * D:8 * D + (c + 1) * D, :].T
        wkv_c = np.ascontiguousarray(
            np.concatenate([kT, vT], axis=1)).astype(bf16)             # [E, 256]
        biases = np.zeros((6, 128), np.float32)
        biases[0:4, :] = w_q_b[c * 512:(c + 1) * 512].reshape(4, 128)
        biases[4, :] = w_kv_b[c * D:(c + 1) * D]
        biases[5, :] = w_kv_b[8 * D + c * D:8 * D + (c + 1) * D]
        pjt_c = np.ascontiguousarray(
            proj_w[:, c * 512:(c + 1) * 512].T).astype(bf16)           # [512, E]
        in_maps.append({
            "xT": xT, "wqT": wq_c, "wkvT": wkv_c, "biases": biases,
            "cosE": cosE, "sinS": sinS, "triM": triM, "sel16": sel16,
            "projT": pjt_c, "ident": ident,
        })

    res = run_bass_kernel_spmd(_PROG, in_maps, core_ids=list(range(NCORES)))
    out = np.zeros((BT, E), np.float32)
    for c in range(NCORES):
        out += res.results[c]["yp"].astype(np.float32)
    out = out + proj_b[None, :].astype(np.float32)
    return out.reshape(B, T, E).astype(np.float32)


# revision 18
# speedup vs baseline: 1.0808x; 1.0020x over previous
"""Llama GQA attention block, tensor-parallel over heads across 8 TRN2 NeuronCores.

Contract: kernel(**inputs) takes the FULL inputs of the reference
(x, freq_cos, freq_sin, w_q_w, w_q_b, w_kv_w, w_kv_b, proj_w, proj_b, start_pos)
and returns the FULL output (B, T, N_EMBD) float32.

Sharding: core c owns query heads 4c..4c+3 and KV head c, plus proj rows
c*512..(c+1)*512. Each core computes a partial projection output (fp16); the
host sums the 8 partials and adds proj_b.

v2 design (vs the 920us baseline):
- softmax row sums no longer use a per-kc ones-matmul on the PE (was 164K
  cycles/core).  exp tiles accumulate into a bf16 SBUF accumulator on the DVE
  (2x packed mode); a single 4-column selector matmul per (b,j) then reduces
  the 128 key-residues, yielding a [4,512] sums tile (one bank, all 4 heads).
- causal trimming: in diagonal 512-blocks the score/AV matmuls and the DVE
  accumulation only cover the valid query range (moving-dim slice); only the
  leading 128-wide triangle needs a mask multiply (GPSIMD, [128,128]).
- both heads of a pair share one 2-bank PSUM score tile [128,1024]; exp runs
  as a single ACTIVATE over 1024 columns (amortizes the ~293ns ACT overhead).
- x is loaded in 4-e-chunk granules (one 512KB DMA for [128,4,512]) and
  output tiles are stored 4-at-a-time (512KB DMAs), cutting HWDGE queue
  dispatch time ~4x and removing the pass-1 chunk-boundary DMA bubbles.
- out-projection stays a PE filler stream popped during attention (exp on the
  ACT engine is the per-kc critical path; filler keeps the PE saturated).
"""

import math
import numpy as np
from contextlib import ExitStack

# Problem constants (hardcoded per the harness contract).
B = 2
T = 2048
E = 4096
D = 128          # head dim
NCORES = 8
HPC = 4          # query heads per core
BT = B * T       # 4096
SQ = 512         # token chunk (matmul moving dim)
ECH = E // 128   # 32 contraction chunks
GE = 4           # e-chunks per x/weight DMA granule
NG = ECH // GE   # granules per token chunk
CPB = T // SQ    # 4 tok chunks per batch
INV_SQRT_D = 1.0 / math.sqrt(D)
HB = 1024        # half-batch token span for the output projection stage


def _build_program():
    import concourse.bass as bass  # noqa: F401
    import concourse.bass_isa as bass_isa
    import concourse.mybir as mybir
    import concourse.tile as tile
    from concourse import bacc

    f32 = mybir.dt.float32
    bf16 = mybir.dt.bfloat16
    f16 = mybir.dt.float16
    AF = mybir.ActivationFunctionType

    nc = bacc.Bacc("TRN2", target_bir_lowering=False, debug=False)

    xT_d = nc.dram_tensor("xT", [E, BT], bf16, kind="ExternalInput")
    wq_d = nc.dram_tensor("wqT", [E, HPC * D], bf16, kind="ExternalInput")
    wkv_d = nc.dram_tensor("wkvT", [E, 2 * D], bf16, kind="ExternalInput")
    bias_d = nc.dram_tensor("biases", [6, 128], f32, kind="ExternalInput")
    cos_d = nc.dram_tensor("cosE", [128, T], bf16, kind="ExternalInput")
    sin_d = nc.dram_tensor("sinS", [128, T], bf16, kind="ExternalInput")
    tri_d = nc.dram_tensor("triM", [128, 128], bf16, kind="ExternalInput")
    sel_d = nc.dram_tensor("sel16", [128, 16], bf16, kind="ExternalInput")
    pjt_d = nc.dram_tensor("projT", [HPC * D, E], bf16, kind="ExternalInput")
    idn_d = nc.dram_tensor("ident", [128, 128], f32, kind="ExternalInput")
    out_d = nc.dram_tensor("yp", [BT, E], f16, kind="ExternalOutput")

    with tile.TileContext(nc) as tc, ExitStack() as ctx:
        const = ctx.enter_context(tc.tile_pool(name="const", bufs=1))
        wpool = ctx.enter_context(tc.tile_pool(name="wpool", bufs=1))
        big = ctx.enter_context(tc.tile_pool(name="big", bufs=1))
        xpool = ctx.enter_context(tc.tile_pool(name="xpool", bufs=3))
        espool = ctx.enter_context(tc.tile_pool(name="espool", bufs=3))
        accp = ctx.enter_context(tc.tile_pool(name="accp", bufs=2))
        spool = ctx.enter_context(tc.tile_pool(name="spool", bufs=2))
        ppool = ctx.enter_context(tc.tile_pool(name="ppool", bufs=2))
        ypool = ctx.enter_context(tc.tile_pool(name="ypool", bufs=2))
        psum = ctx.enter_context(tc.tile_pool(name="ps", bufs=1, space="PSUM"))

        # ---- weights / constants resident in SBUF ----
        wq_sb = wpool.tile([128, ECH, HPC * D], bf16, tag="wq")
        wkv_sb = wpool.tile([128, ECH, 2 * D], bf16, tag="wkv")
        # granule loads on the scalar HWDGE ring (x rides the sync ring).
        # wq/wkv granules interleave so the first k/v matmuls aren't stuck
        # behind all 4MB of wq; the leading granules are small for fast start.
        WGRAN = [(0, 1), (1, 1), (2, 2), (4, 4)]
        for e0, ge in WGRAN:
            nc.scalar.dma_start(
                wq_sb[:, e0:e0 + ge, :],
                wq_d[e0 * 128:(e0 + ge) * 128, :]
                .rearrange("(f p) c -> p f c", p=128))
            nc.scalar.dma_start(
                wkv_sb[:, e0:e0 + ge, :],
                wkv_d[e0 * 128:(e0 + ge) * 128, :]
                .rearrange("(f p) c -> p f c", p=128))
        bias_sb = const.tile([128, 6], f32, tag="bias")
        cos_sb = const.tile([128, T], bf16, tag="cos")
        sin_sb = const.tile([128, T], bf16, tag="sin")
        tri_sb = const.tile([128, 128], bf16, tag="tri")
        sel_sb = const.tile([128, 16], bf16, tag="sel")
        idn_sb = const.tile([128, 128], f32, tag="idn")
        pjt_sb = wpool.tile([128, HPC, E], bf16, tag="pjt")

        def load_consts():
            nc.scalar.dma_start(bias_sb[:], bias_d.rearrange("r p -> p r"))
            nc.scalar.dma_start(cos_sb[:], cos_d[:, :])
            nc.scalar.dma_start(sin_sb[:], sin_d[:, :])
            nc.scalar.dma_start(idn_sb[:], idn_d[:, :])
            nc.scalar.dma_start(sel_sb[:], sel_d[:, :])
            nc.scalar.dma_start(tri_sb[:], tri_d[:, :])

        # big SBUF-resident intermediates (bf16): rotated Q, rotated K, V^T
        qrot = big.tile([128, HPC, BT], bf16, tag="qrot")   # [d, h, tok]
        rotK = big.tile([128, BT], bf16, tag="rotK")        # [d, tok]
        vbufT = big.tile([128, BT], bf16, tag="vbuf")       # [tok%128, kc*128+d]

        # PSUM layout (single pool, tags in declared order -> banks):
        #   sc0 [128,1024] banks 0-1 | sc1 [128,1024] banks 2-3
        #   yt0 bank 4 | yt1 bank 5 | po0 bank 6 | po1 bank 7
        sc_tag = ["sc0", "sc1"]
        tp_alt = 0      # V-transpose bank alternator (po0/po1)
        po_alt = 0      # out-proj bank alternator (po0/po1)
        po_ct = 0       # out-proj eviction engine pattern counter

        # ---------------- pass 1: QKV projection + rope ----------------
        # V transposes are deferred into the NEXT chunk's matmul stream so
        # the PE never waits on the ScalarE eviction chain at chunk edges.
        pending_v = []

        def flush_v():
            nonlocal tp_alt
            while pending_v:
                vraw_p, gcol_p = pending_v.pop(0)
                for t4 in range(4):
                    tp = psum.tile([128, 128], f32,
                                   tag=["po0", "po1"][tp_alt],
                                   name=f"tp_{gcol_p}_{t4}")
                    tp_alt ^= 1
                    nc.tensor.transpose(
                        tp[:], vraw_p[:, t4 * 128:(t4 + 1) * 128], idn_sb[:])
                    nc.scalar.copy(vbufT[:, gcol_p + t4 * 128:
                                         gcol_p + (t4 + 1) * 128], tp[:])

        for b in range(B):
            for j in range(CPB):
                gcol = b * T + j * SQ
                tcol = j * SQ
                acc01 = psum.tile([128, 1024], f32, tag="sc0",
                                  name=f"acc01_{b}_{j}")
                acc23 = psum.tile([128, 1024], f32, tag="sc1",
                                  name=f"acc23_{b}_{j}")
                acck = psum.tile([128, SQ], f32, tag="yt0",
                                 name=f"acck_{b}_{j}")
                accv = psum.tile([128, SQ], f32, tag="yt1",
                                 name=f"accv_{b}_{j}")
                accs = [acc01[:, 0:512], acc01[:, 512:1024],
                        acc23[:, 0:512], acc23[:, 512:1024]]
                xplan = ([(0, 1), (1, 1), (2, 2)]
                         + [(e0, GE) for e0 in range(4, ECH, GE)]
                         if (b == 0 and j == 0) else
                         [(e0, GE) for e0 in range(0, ECH, GE)])
                for e0, ge in xplan:
                    xt = xpool.tile([128, ge, SQ], bf16, tag="xt")
                    nc.sync.dma_start(
                        xt[:],
                        xT_d[e0 * 128:(e0 + ge) * 128, gcol:gcol + SQ]
                        .rearrange("(f p) t -> p f t", p=128))
                    for f in range(ge):
                        e = e0 + f
                        st, sp = (e == 0), (e == ECH - 1)
                        for h in range(HPC):
                            nc.tensor.matmul(
                                accs[h], wq_sb[:, e, h * D:(h + 1) * D],
                                xt[:, f, :], start=st, stop=sp)
                        nc.tensor.matmul(acck[:], wkv_sb[:, e, 0:D],
                                         xt[:, f, :], start=st, stop=sp)
                        nc.tensor.matmul(accv[:], wkv_sb[:, e, D:2 * D],
                                         xt[:, f, :], start=st, stop=sp)
                    if b == 0 and j == 0 and e0 >= 4 and e0 < ECH - 4:
                        # just-in-time weight granules: keep early HBM
                        # bandwidth free for the x stream
                        we = e0 + 4
                        nc.scalar.dma_start(
                            wq_sb[:, we:we + GE, :],
                            wq_d[we * 128:(we + GE) * 128, :]
                            .rearrange("(f p) c -> p f c", p=128))
                        nc.scalar.dma_start(
                            wkv_sb[:, we:we + GE, :],
                            wkv_d[we * 128:(we + GE) * 128, :]
                            .rearrange("(f p) c -> p f c", p=128))
                    if e0 == 8:
                        # keep the LAST chunk's predecessor un-flushed too:
                        # both tail chunks' V transposes become pass-2 filler
                        if not (b == 1 and j == 3):
                            flush_v()
                        if b == 0 and j == 0:
                            load_consts()
                    if b == 1 and j == 2 and e0 == 8:
                        for hq in range(HPC):
                            nc.scalar.dma_start(
                                pjt_sb[:, hq, :],
                                pjt_d[hq * 128:(hq + 1) * 128, :])

                # two-phase rope: evictions split ScalarE (q0,q2,V) /
                # VectorE (q1,q3,K); VEC-evicted heads swap rope pairs via
                # stream_shuffle, ScalarE-evicted ones via strided DMA copies.
                SWAPM = [a + 1 - 2 * (a % 2) for a in range(32)]
                VECH = (1, 3, 4)
                raws, sws = {}, {}
                for c in (0, 1, 2, 3, 4):
                    src = accs[c] if c < 4 else acck[:]
                    raw = spool.tile([128, SQ], bf16, tag="raw", bufs=5,
                                     name=f"raw_{b}_{j}_{c}")
                    if c in VECH:
                        with nc.allow_low_precision(reason="rope evict"):
                            nc.vector.tensor_add(
                                raw[:], src,
                                bias_sb[:, c:c + 1].to_broadcast((128, SQ)))
                    else:
                        nc.scalar.activation(
                            raw[:], src, AF.Identity,
                            bias=bias_sb[:, c:c + 1])
                    raws[c] = raw
                vraw = spool.tile([128, SQ], f32, tag="vraw", bufs=2,
                                  name=f"vraw_{b}_{j}")
                nc.scalar.activation(vraw[:], accv[:], AF.Identity,
                                     bias=bias_sb[:, 5:6])
                pending_v.append((vraw, gcol))
                for c in (0, 1, 2, 3, 4):
                    sw = spool.tile([128, SQ], bf16, tag="sw", bufs=5,
                                    name=f"sw_{b}_{j}_{c}")
                    if c in VECH:
                        nc.vector.stream_shuffle(sw[:], raws[c][:], SWAPM)
                    else:
                        raw3 = raws[c].rearrange("(a two) t -> a two t", two=2)
                        sw3 = sw.rearrange("(a two) t -> a two t", two=2)
                        nc.scalar.dma_start(sw3[:, 1, :], raw3[:, 0, :])
                        nc.scalar.dma_start(sw3[:, 0, :], raw3[:, 1, :])
                    sws[c] = sw
                for c in (0, 1, 2, 3, 4):
                    out_ap = (qrot[:, c, gcol:gcol + SQ] if c < 4
                              else rotK[:, gcol:gcol + SQ])
                    tmp = spool.tile([128, SQ], bf16, tag="rtmp", bufs=3,
                                     name=f"rtmp_{b}_{j}_{c}")
                    nc.vector.tensor_mul(tmp[:], raws[c][:],
                                         cos_sb[:, tcol:tcol + SQ])
                    nc.vector.tensor_mul(sws[c][:], sws[c][:],
                                         sin_sb[:, tcol:tcol + SQ])
                    nc.vector.tensor_add(out_ap, tmp[:], sws[c][:])
        # the last chunk's V transposes are emitted at the start of pass 2

        # ------------- pass 2: attention + output projection -------------
        pend = []   # (kind, block, closure) pending items (PE filler)

        drain = [False]

        def po_group(yts_t, grow, oc, ts8, postg_t, fslot):
            nonlocal po_alt, po_ct
            # post-attention drain: all 8 banks are free -> rotate over 4
            # and split evictions 50/50 so the PE never waits on a bank.
            tags = (["po0", "po1", "yt0", "yt1"] if drain[0]
                    else ["po0", "po1"])
            po_ps = psum.tile([128, SQ], f32, tag=tags[po_alt % len(tags)],
                              name=f"pops_{grow}_{oc}_{ts8}")
            po_alt += 1
            for h in range(HPC):
                nc.tensor.matmul(
                    po_ps[:], yts_t[:, h, ts8 * 128:(ts8 + 1) * 128],
                    pjt_sb[:, h, oc * SQ:(oc + 1) * SQ],
                    start=(h == 0), stop=(h == HPC - 1))
            act_turn = (po_ct % 2 == 1) if drain[0] else (po_ct % 3 == 1)
            if act_turn:
                nc.scalar.copy(postg_t[:, fslot, :], po_ps[:])
            else:
                nc.vector.tensor_copy(postg_t[:, fslot, :], po_ps[:])
            po_ct += 1

        def po_store(postg_t, grow, oc, f0):
            nc.sync.dma_start(
                out_d[grow + f0 * 128:grow + (f0 + 4) * 128,
                      oc * SQ:(oc + 1) * SQ]
                .rearrange("(f p) t -> p f t", p=128),
                postg_t[:])

        def pop_pend(n, reserve=0):
            for _ in range(n):
                if len(pend) <= reserve:
                    return
                pend.pop(0)[2]()

        def pop_stale_norms(cur_blk):
            # emit any previous block's norm_muls (their sums chains are long
            # done) before this block's first write to the bufs=1 yty slot
            while any(k == "norm" and bl < cur_blk for k, bl, _ in pend):
                pend.pop(0)[2]()

        # the last pass-1 chunk's V transposes become the first PE filler
        # items (pass-2 j=0 has no out-proj work yet); evictions alternate
        # VectorE/ScalarE so the exp stream isn't delayed on ACT.
        def v_tr_one(vraw_p, gcol_p, t4, on_vec):
            nonlocal tp_alt
            tp = psum.tile([128, 128], f32, tag=["po0", "po1"][tp_alt],
                           name=f"tpf_{gcol_p}_{t4}")
            tp_alt ^= 1
            nc.tensor.transpose(
                tp[:], vraw_p[:, t4 * 128:(t4 + 1) * 128], idn_sb[:])
            dst = vbufT[:, gcol_p + t4 * 128:gcol_p + (t4 + 1) * 128]
            if on_vec:
                nc.vector.tensor_copy(dst, tp[:])
            else:
                nc.scalar.copy(dst, tp[:])
        while pending_v:
            vraw_p, gcol_p = pending_v.pop(0)
            for t4 in range(4):
                pend.append(("po", -1,
                             lambda v=vraw_p, g=gcol_p, t=t4, ov=(t4 % 2 == 0):
                             v_tr_one(v, g, t, ov)))

        def norm_mul(yts_t, yty_t, rr0_t, jj_, pr_, i_):
            h = pr_ * 2 + i_
            sidx = jj_ * 4 + h
            rb = spool.tile([128, SQ], bf16, tag="rb", bufs=2,
                            name=f"rb_{sidx}")
            nc.gpsimd.partition_broadcast(
                rb[:], rr0_t[0:1, i_, :], channels=128)
            nc.vector.tensor_mul(
                yts_t[:, h, jj_ * SQ:(jj_ + 1) * SQ],
                yty_t[:, sidx, :], rb[:])

        blk_ct = [0]

        for b in range(B):
            # process j-groups densest-first (j=3..0): the first group runs
            # with an empty filler backlog, so give it the best PE/exp ratio;
            # j=0 (mostly-masked, thin PE) runs last against a deep backlog.
            for hh in (1, 0):
                blk = blk_ct[0]
                blk_ct[0] += 1
                need_norm_drain = [True]
                yts = ypool.tile([128, HPC, HB], bf16, tag="yts",
                                 name=f"yts_{b}_{hh}")
                # reuse the (pass-1-only) wkv weight slot for the
                # unnormalized attention staging buffer: same pool tag ->
                # same SBUF bytes, WAR deps order it after the last QKV read.
                yty = wpool.tile([128, 8, SQ], bf16, tag="wkv", bufs=1,
                                 name=f"yty_{b}_{hh}")
                for jj in (1, 0):
                    j = hh * 2 + jj
                    last_jj = (b == 1 and hh == 0 and jj == 0)
                    gcol = b * T + j * SQ
                    nkc = 4 * j + 4
                    for pr in range(2):          # head pairs (2h per pass)
                        hs = (2 * pr, 2 * pr + 1)
                        yt_ps = [psum.tile([128, SQ], f32,
                                           tag=["yt0", "yt1"][i],
                                           name=f"yt_{b}_{j}_{h}")
                                 for i, h in enumerate(hs)]
                        acc2 = accp.tile([128, 1024], bf16, tag="acc",
                                         name=f"acc2_{b}_{j}_{pr}")
                        prev = None

                        def emit_av(kc, es_t, off):
                            st, sp = (kc == 0), (kc == nkc - 1)
                            koff = b * T + kc * 128
                            for i in range(2):
                                nc.tensor.matmul(
                                    yt_ps[i][:, off:SQ],
                                    vbufT[:, koff:koff + 128],
                                    es_t[:, i * SQ + off:(i + 1) * SQ],
                                    start=st, stop=sp)

                        for kc in range(nkc):
                            koff = b * T + kc * 128
                            off = 128 * max(0, kc - 4 * j)
                            s2 = psum.tile([128, 1024], f32,
                                           tag=sc_tag[kc % 2],
                                           name=f"s_{b}_{j}_{kc}_{pr}")
                            for i, h in enumerate(hs):
                                nc.tensor.matmul(
                                    s2[:, i * SQ + off:(i + 1) * SQ],
                                    rotK[:, koff:koff + 128],
                                    qrot[:, h, gcol + off:gcol + SQ],
                                    start=True, stop=True)
                            es = espool.tile([128, 1024], bf16, tag="es",
                                             name=f"es_{b}_{j}_{kc}_{pr}")
                            nc.scalar.activation(es[:], s2[:],
                                                 AF.Exp, scale=INV_SQRT_D)
                            if off > 0 or kc == 4 * j:
                                # mask the leading 128-wide causal triangle
                                # (DVE, NOT gpsimd: mixing ops on gpsimd
                                # thrashes its loadable Q7 library against
                                # partition_broadcast, ~6us per swap)
                                for i in range(2):
                                    nc.vector.tensor_mul(
                                        es[:, i * SQ + off:i * SQ + off + 128],
                                        es[:, i * SQ + off:i * SQ + off + 128],
                                        tri_sb[:])
                            # softmax denominators: accumulate exp on DVE
                            if kc == 0:
                                nc.vector.tensor_copy(acc2[:], es[:])
                            elif off == 0:
                                nc.vector.tensor_add(acc2[:], acc2[:], es[:])
                            else:
                                for i in range(2):
                                    nc.vector.tensor_add(
                                        acc2[:, i * SQ + off:(i + 1) * SQ],
                                        acc2[:, i * SQ + off:(i + 1) * SQ],
                                        es[:, i * SQ + off:(i + 1) * SQ])
                            if prev is not None:
                                emit_av(kc - 1, *prev)
                            prev = (es, off)
                            pop_pend(2, reserve=20)
                        emit_av(nkc - 1, *prev)

                        if need_norm_drain[0]:
                            pop_stale_norms(blk)
                            need_norm_drain[0] = False
                        # stage attention outputs out of PSUM right away
                        for i, h in enumerate(hs):
                            sidx = jj * 4 + pr * 2 + i
                            nc.vector.tensor_copy(yty[:, sidx, :],
                                                  yt_ps[i][:])
                        pop_pend(2)
                        # per-pair softmax sums: 2 selector matmuls reduce the
                        # bf16 accumulator's 128 key-residues; the whole
                        # reciprocal chain stays on DVE (no ACT-queue hops)
                        sums2 = psum.tile([2, SQ], f32, tag="yt0",
                                          name=f"sums2_{b}_{j}_{pr}")
                        for i in range(2):
                            nc.tensor.matmul(
                                sums2[:], sel_sb[:, 4 * i:4 * i + 2],
                                acc2[:, i * SQ:(i + 1) * SQ],
                                start=(i == 0), stop=(i == 1))
                        stg = spool.tile([2, SQ], f32, tag="vraw", bufs=2,
                                         name=f"stg_{b}_{j}_{pr}")
                        nc.vector.tensor_copy(stg[:], sums2[:])
                        rrf = spool.tile([2, SQ], f32, tag="vraw", bufs=2,
                                         name=f"rrf_{b}_{j}_{pr}")
                        nc.vector.reciprocal_approx_fast(rrf[:], stg[:])
                        rrb = spool.tile([2, SQ], bf16, tag="rrb", bufs=2,
                                         name=f"rrb_{b}_{j}_{pr}")
                        with nc.allow_low_precision(reason="softmax recip"):
                            nc.vector.tensor_copy(rrb[:], rrf[:])
                        # partition_broadcast only reads partition 0: hop the
                        # rows there via tiny SBUF->SBUF DMAs (idle sync ring)
                        rr0 = spool.tile([1, 2, SQ], bf16, tag="rr0", bufs=2,
                                         name=f"rr0_{b}_{j}_{pr}")
                        for i in range(2):
                            nc.sync.dma_start(rr0[0:1, i, :], rrb[i:i + 1, :])
                        for i in range(2):
                            pend.append(("norm", blk,
                                         lambda y=yts, yy=yty, dd=rr0,
                                         a=jj, p=pr, q=i:
                                         norm_mul(y, yy, dd, a, p, q)))

                    grow = b * T + hh * HB
                    for oc in range(8):
                        postg = ppool.tile([128, 4, SQ], f16, tag="po",
                                           name=f"postg_{b}_{hh}_{jj}_{oc}")
                        for k4, ts8 in enumerate(range(jj * 4, jj * 4 + 4)):
                            pend.append(("po", blk,
                                         lambda y=yts, g=grow, o=oc, t=ts8,
                                         pt=postg, k=k4:
                                         po_group(y, g, o, t, pt, k)))
                        pend.append(("po", blk,
                                     lambda pt=postg, g=grow, o=oc, f0=jj * 4:
                                     po_store(pt, g, o, f0)))
        drain[0] = True
        pop_pend(len(pend))

    nc.compile()
    return nc


_PROG = None


def kernel(x, freq_cos, freq_sin, w_q_w, w_q_b, w_kv_w, w_kv_b, proj_w, proj_b,
           start_pos=0, **_unused):
    global _PROG
    import ml_dtypes
    from concourse.bass_utils import run_bass_kernel_spmd

    bf16 = ml_dtypes.bfloat16

    x = np.asarray(x, np.float32)
    freq_cos = np.asarray(freq_cos, np.float32)
    freq_sin = np.asarray(freq_sin, np.float32)
    w_q_w = np.asarray(w_q_w, np.float32)
    w_q_b = np.asarray(w_q_b, np.float32)
    w_kv_w = np.asarray(w_kv_w, np.float32)
    w_kv_b = np.asarray(w_kv_b, np.float32)
    proj_w = np.asarray(proj_w, np.float32)
    proj_b = np.asarray(proj_b, np.float32)

    xT = np.ascontiguousarray(x.reshape(BT, E).T).astype(bf16)

    cosE = np.repeat(freq_cos.T, 2, axis=0).astype(np.float32)        # [128, T]
    sinE = np.repeat(freq_sin.T, 2, axis=0).astype(np.float32)
    sinS = sinE.copy()
    sinS[0::2, :] *= -1.0                                             # even rows -sin
    cosE = cosE.astype(bf16)
    sinS = sinS.astype(bf16)

    kp = np.arange(128)[:, None]
    qq = np.arange(128)[None, :]
    triM = (qq >= kp).astype(bf16)                                    # [128, 128]

    sel16 = np.zeros((128, 16), np.float32)
    for h in range(4):
        sel16[:, 4 * h + h] = 1.0
    sel16 = sel16.astype(bf16)

    ident = np.eye(128, dtype=np.float32)

    if _PROG is None:
        _PROG = _build_program()

    in_maps = []
    for c in range(NCORES):
        wq_c = np.ascontiguousarray(
            w_q_w[c * 512:(c + 1) * 512, :].T).astype(bf16)            # [E, 512]
        kT = w_kv_w[c * D:(c + 1) * D, :].T                            # [E, 128]
        vT = w_kv_w[8 * D + c * D:8 * D + (c + 1) * D, :].T
        wkv_c = np.ascontiguousarray(
            np.concatenate([kT, vT], axis=1)).astype(bf16)             # [E, 256]
        biases = np.zeros((6, 128), np.float32)
        biases[0:4, :] = w_q_b[c * 512:(c + 1) * 512].reshape(4, 128)
        biases[4, :] = w_kv_b[c * D:(c + 1) * D]
        biases[5, :] = w_kv_b[8 * D + c * D:8 * D + (c + 1) * D]
        pjt_c = np.ascontiguousarray(
            proj_w[:, c * 512:(c + 1) * 512].T).astype(bf16)           # [512, E]
        in_maps.append({
            "xT": xT, "wqT": wq_c, "wkvT": wkv_c, "biases": biases,
            "cosE": cosE, "sinS": sinS, "triM": triM, "sel16": sel16,
            "projT": pjt_c, "ident": ident,
        })

    res = run_bass_kernel_spmd(_PROG, in_maps, core_ids=list(range(NCORES)))
    out = np.zeros((BT, E), np.float32)
    for c in range(NCORES):
        out += res.results[c]["yp"].astype(np.float32)
    out = out + proj_b[None, :].astype(np.float32)
    return out.reshape(B, T, E).astype(np.float32)


# revision 19
# speedup vs baseline: 1.0936x; 1.0119x over previous
"""Llama GQA attention block, tensor-parallel over heads across 8 TRN2 NeuronCores.

Contract: kernel(**inputs) takes the FULL inputs of the reference
(x, freq_cos, freq_sin, w_q_w, w_q_b, w_kv_w, w_kv_b, proj_w, proj_b, start_pos)
and returns the FULL output (B, T, N_EMBD) float32.

Sharding: core c owns query heads 4c..4c+3 and KV head c, plus proj rows
c*512..(c+1)*512. Each core computes a partial projection output (fp16); the
host sums the 8 partials and adds proj_b.

v2 design (vs the 920us baseline):
- softmax row sums no longer use a per-kc ones-matmul on the PE (was 164K
  cycles/core).  exp tiles accumulate into a bf16 SBUF accumulator on the DVE
  (2x packed mode); a single 4-column selector matmul per (b,j) then reduces
  the 128 key-residues, yielding a [4,512] sums tile (one bank, all 4 heads).
- causal trimming: in diagonal 512-blocks the score/AV matmuls and the DVE
  accumulation only cover the valid query range (moving-dim slice); only the
  leading 128-wide triangle needs a mask multiply (GPSIMD, [128,128]).
- both heads of a pair share one 2-bank PSUM score tile [128,1024]; exp runs
  as a single ACTIVATE over 1024 columns (amortizes the ~293ns ACT overhead).
- x is loaded in 4-e-chunk granules (one 512KB DMA for [128,4,512]) and
  output tiles are stored 4-at-a-time (512KB DMAs), cutting HWDGE queue
  dispatch time ~4x and removing the pass-1 chunk-boundary DMA bubbles.
- out-projection stays a PE filler stream popped during attention (exp on the
  ACT engine is the per-kc critical path; filler keeps the PE saturated).
"""

import math
import numpy as np
from contextlib import ExitStack

# Problem constants (hardcoded per the harness contract).
B = 2
T = 2048
E = 4096
D = 128          # head dim
NCORES = 8
HPC = 4          # query heads per core
BT = B * T       # 4096
SQ = 512         # token chunk (matmul moving dim)
ECH = E // 128   # 32 contraction chunks
GE = 2           # e-chunks per x/weight DMA granule
NG = ECH // GE   # granules per token chunk
CPB = T // SQ    # 4 tok chunks per batch
INV_SQRT_D = 1.0 / math.sqrt(D)
HB = 1024        # half-batch token span for the output projection stage


def _build_program():
    import concourse.bass as bass  # noqa: F401
    import concourse.bass_isa as bass_isa
    import concourse.mybir as mybir
    import concourse.tile as tile
    from concourse import bacc

    f32 = mybir.dt.float32
    bf16 = mybir.dt.bfloat16
    f16 = mybir.dt.float16
    AF = mybir.ActivationFunctionType

    nc = bacc.Bacc("TRN2", target_bir_lowering=False, debug=False)

    xT_d = nc.dram_tensor("xT", [E, BT], bf16, kind="ExternalInput")
    wq_d = nc.dram_tensor("wqT", [E, HPC * D], bf16, kind="ExternalInput")
    wkv_d = nc.dram_tensor("wkvT", [E, 2 * D], bf16, kind="ExternalInput")
    bias_d = nc.dram_tensor("biases", [6, 128], f32, kind="ExternalInput")
    cos_d = nc.dram_tensor("cosE", [128, T], bf16, kind="ExternalInput")
    sin_d = nc.dram_tensor("sinS", [128, T], bf16, kind="ExternalInput")
    tri_d = nc.dram_tensor("triM", [128, 128], bf16, kind="ExternalInput")
    sel_d = nc.dram_tensor("sel16", [128, 16], bf16, kind="ExternalInput")
    pjt_d = nc.dram_tensor("projT", [HPC * D, E], bf16, kind="ExternalInput")
    idn_d = nc.dram_tensor("ident", [128, 128], f32, kind="ExternalInput")
    out_d = nc.dram_tensor("yp", [BT, E], f16, kind="ExternalOutput")

    with tile.TileContext(nc) as tc, ExitStack() as ctx:
        const = ctx.enter_context(tc.tile_pool(name="const", bufs=1))
        wpool = ctx.enter_context(tc.tile_pool(name="wpool", bufs=1))
        big = ctx.enter_context(tc.tile_pool(name="big", bufs=1))
        xpool = ctx.enter_context(tc.tile_pool(name="xpool", bufs=6))
        espool = ctx.enter_context(tc.tile_pool(name="espool", bufs=3))
        accp = ctx.enter_context(tc.tile_pool(name="accp", bufs=2))
        spool = ctx.enter_context(tc.tile_pool(name="spool", bufs=2))
        ppool = ctx.enter_context(tc.tile_pool(name="ppool", bufs=2))
        ypool = ctx.enter_context(tc.tile_pool(name="ypool", bufs=2))
        psum = ctx.enter_context(tc.tile_pool(name="ps", bufs=1, space="PSUM"))

        # ---- weights / constants resident in SBUF ----
        wq_sb = wpool.tile([128, ECH, HPC * D], bf16, tag="wq")
        wkv_sb = wpool.tile([128, ECH, 2 * D], bf16, tag="wkv")
        # granule loads on the scalar HWDGE ring (x rides the sync ring).
        # wq/wkv granules interleave so the first k/v matmuls aren't stuck
        # behind all 4MB of wq; the leading granules are small for fast start.
        WGRAN = [(0, 1), (1, 1), (2, 2), (4, 4)]
        for e0, ge in WGRAN:
            nc.scalar.dma_start(
                wq_sb[:, e0:e0 + ge, :],
                wq_d[e0 * 128:(e0 + ge) * 128, :]
                .rearrange("(f p) c -> p f c", p=128))
            nc.scalar.dma_start(
                wkv_sb[:, e0:e0 + ge, :],
                wkv_d[e0 * 128:(e0 + ge) * 128, :]
                .rearrange("(f p) c -> p f c", p=128))
        bias_sb = const.tile([128, 6], f32, tag="bias")
        cos_sb = const.tile([128, T], bf16, tag="cos")
        sin_sb = const.tile([128, T], bf16, tag="sin")
        tri_sb = const.tile([128, 128], bf16, tag="tri")
        sel_sb = const.tile([128, 16], bf16, tag="sel")
        idn_sb = const.tile([128, 128], f32, tag="idn")
        pjt_sb = wpool.tile([128, HPC, E], bf16, tag="pjt")

        def load_consts():
            nc.scalar.dma_start(bias_sb[:], bias_d.rearrange("r p -> p r"))
            nc.scalar.dma_start(cos_sb[:], cos_d[:, :])
            nc.scalar.dma_start(sin_sb[:], sin_d[:, :])
            nc.scalar.dma_start(idn_sb[:], idn_d[:, :])
            nc.scalar.dma_start(sel_sb[:], sel_d[:, :])
            nc.scalar.dma_start(tri_sb[:], tri_d[:, :])

        # big SBUF-resident intermediates (bf16): rotated Q, rotated K, V^T
        qrot = big.tile([128, HPC, BT], bf16, tag="qrot")   # [d, h, tok]
        rotK = big.tile([128, BT], bf16, tag="rotK")        # [d, tok]
        vbufT = big.tile([128, BT], bf16, tag="vbuf")       # [tok%128, kc*128+d]

        # PSUM layout (single pool, tags in declared order -> banks):
        #   sc0 [128,1024] banks 0-1 | sc1 [128,1024] banks 2-3
        #   yt0 bank 4 | yt1 bank 5 | po0 bank 6 | po1 bank 7
        sc_tag = ["sc0", "sc1"]
        tp_alt = 0      # V-transpose bank alternator (po0/po1)
        po_alt = 0      # out-proj bank alternator (po0/po1)
        po_ct = 0       # out-proj eviction engine pattern counter

        # ---------------- pass 1: QKV projection + rope ----------------
        # V transposes are deferred into the NEXT chunk's matmul stream so
        # the PE never waits on the ScalarE eviction chain at chunk edges.
        pending_v = []

        def flush_v():
            nonlocal tp_alt
            while pending_v:
                vraw_p, gcol_p = pending_v.pop(0)
                for t4 in range(4):
                    tp = psum.tile([128, 128], f32,
                                   tag=["po0", "po1"][tp_alt],
                                   name=f"tp_{gcol_p}_{t4}")
                    tp_alt ^= 1
                    nc.tensor.transpose(
                        tp[:], vraw_p[:, t4 * 128:(t4 + 1) * 128], idn_sb[:])
                    nc.scalar.copy(vbufT[:, gcol_p + t4 * 128:
                                         gcol_p + (t4 + 1) * 128], tp[:])

        for b in range(B):
            for j in range(CPB):
                gcol = b * T + j * SQ
                tcol = j * SQ
                acc01 = psum.tile([128, 1024], f32, tag="sc0",
                                  name=f"acc01_{b}_{j}")
                acc23 = psum.tile([128, 1024], f32, tag="sc1",
                                  name=f"acc23_{b}_{j}")
                acck = psum.tile([128, SQ], f32, tag="yt0",
                                 name=f"acck_{b}_{j}")
                accv = psum.tile([128, SQ], f32, tag="yt1",
                                 name=f"accv_{b}_{j}")
                accs = [acc01[:, 0:512], acc01[:, 512:1024],
                        acc23[:, 0:512], acc23[:, 512:1024]]
                xplan = ([(0, 1), (1, 1)]
                         + [(e0, GE) for e0 in range(2, ECH, GE)]
                         if (b == 0 and j == 0) else
                         [(e0, GE) for e0 in range(0, ECH, GE)])
                for e0, ge in xplan:
                    xt = xpool.tile([128, ge, SQ], bf16, tag="xt")
                    nc.sync.dma_start(
                        xt[:],
                        xT_d[e0 * 128:(e0 + ge) * 128, gcol:gcol + SQ]
                        .rearrange("(f p) t -> p f t", p=128))
                    for f in range(ge):
                        e = e0 + f
                        st, sp = (e == 0), (e == ECH - 1)
                        for h in range(HPC):
                            nc.tensor.matmul(
                                accs[h], wq_sb[:, e, h * D:(h + 1) * D],
                                xt[:, f, :], start=st, stop=sp)
                        nc.tensor.matmul(acck[:], wkv_sb[:, e, 0:D],
                                         xt[:, f, :], start=st, stop=sp)
                        nc.tensor.matmul(accv[:], wkv_sb[:, e, D:2 * D],
                                         xt[:, f, :], start=st, stop=sp)
                    if b == 0 and j == 0 and e0 >= 4 and e0 < ECH - 4:
                        # just-in-time weight granules: keep early HBM
                        # bandwidth free for the x stream
                        we = e0 + 4
                        nc.scalar.dma_start(
                            wq_sb[:, we:we + GE, :],
                            wq_d[we * 128:(we + GE) * 128, :]
                            .rearrange("(f p) c -> p f c", p=128))
                        nc.scalar.dma_start(
                            wkv_sb[:, we:we + GE, :],
                            wkv_d[we * 128:(we + GE) * 128, :]
                            .rearrange("(f p) c -> p f c", p=128))
                    if e0 == 8:
                        # keep the LAST chunk's predecessor un-flushed too:
                        # both tail chunks' V transposes become pass-2 filler
                        if not (b == 1 and j == 3):
                            flush_v()
                        if b == 0 and j == 0:
                            load_consts()
                    if b == 1 and j == 2 and e0 == 8:
                        for hq in range(HPC):
                            nc.scalar.dma_start(
                                pjt_sb[:, hq, :],
                                pjt_d[hq * 128:(hq + 1) * 128, :])

                # two-phase rope: evictions split ScalarE (q0,q2,V) /
                # VectorE (q1,q3,K); VEC-evicted heads swap rope pairs via
                # stream_shuffle, ScalarE-evicted ones via strided DMA copies.
                SWAPM = [a + 1 - 2 * (a % 2) for a in range(32)]
                VECH = (1, 3, 4)
                raws, sws = {}, {}
                for c in (0, 1, 2, 3, 4):
                    src = accs[c] if c < 4 else acck[:]
                    raw = spool.tile([128, SQ], bf16, tag="raw", bufs=5,
                                     name=f"raw_{b}_{j}_{c}")
                    if c in VECH:
                        with nc.allow_low_precision(reason="rope evict"):
                            nc.vector.tensor_add(
                                raw[:], src,
                                bias_sb[:, c:c + 1].to_broadcast((128, SQ)))
                    else:
                        nc.scalar.activation(
                            raw[:], src, AF.Identity,
                            bias=bias_sb[:, c:c + 1])
                    raws[c] = raw
                vraw = spool.tile([128, SQ], f32, tag="vraw", bufs=2,
                                  name=f"vraw_{b}_{j}")
                nc.scalar.activation(vraw[:], accv[:], AF.Identity,
                                     bias=bias_sb[:, 5:6])
                pending_v.append((vraw, gcol))
                for c in (0, 1, 2, 3, 4):
                    sw = spool.tile([128, SQ], bf16, tag="sw", bufs=5,
                                    name=f"sw_{b}_{j}_{c}")
                    if c in VECH:
                        nc.vector.stream_shuffle(sw[:], raws[c][:], SWAPM)
                    else:
                        raw3 = raws[c].rearrange("(a two) t -> a two t", two=2)
                        sw3 = sw.rearrange("(a two) t -> a two t", two=2)
                        nc.scalar.dma_start(sw3[:, 1, :], raw3[:, 0, :])
                        nc.scalar.dma_start(sw3[:, 0, :], raw3[:, 1, :])
                    sws[c] = sw
                for c in (0, 1, 2, 3, 4):
                    out_ap = (qrot[:, c, gcol:gcol + SQ] if c < 4
                              else rotK[:, gcol:gcol + SQ])
                    tmp = spool.tile([128, SQ], bf16, tag="rtmp", bufs=3,
                                     name=f"rtmp_{b}_{j}_{c}")
                    nc.vector.tensor_mul(tmp[:], raws[c][:],
                                         cos_sb[:, tcol:tcol + SQ])
                    nc.vector.tensor_mul(sws[c][:], sws[c][:],
                                         sin_sb[:, tcol:tcol + SQ])
                    nc.vector.tensor_add(out_ap, tmp[:], sws[c][:])
        # the last chunk's V transposes are emitted at the start of pass 2

        # ------------- pass 2: attention + output projection -------------
        pend = []   # (kind, block, closure) pending items (PE filler)

        drain = [False]

        def po_group(yts_t, grow, oc, ts8, postg_t, fslot):
            nonlocal po_alt, po_ct
            # post-attention drain: all 8 banks are free -> rotate over 4
            # and split evictions 50/50 so the PE never waits on a bank.
            tags = (["po0", "po1", "yt0", "yt1"] if drain[0]
                    else ["po0", "po1"])
            po_ps = psum.tile([128, SQ], f32, tag=tags[po_alt % len(tags)],
                              name=f"pops_{grow}_{oc}_{ts8}")
            po_alt += 1
            for h in range(HPC):
                nc.tensor.matmul(
                    po_ps[:], yts_t[:, h, ts8 * 128:(ts8 + 1) * 128],
                    pjt_sb[:, h, oc * SQ:(oc + 1) * SQ],
                    start=(h == 0), stop=(h == HPC - 1))
            act_turn = (po_ct % 2 == 1) if drain[0] else (po_ct % 3 == 1)
            if act_turn:
                nc.scalar.copy(postg_t[:, fslot, :], po_ps[:])
            else:
                nc.vector.tensor_copy(postg_t[:, fslot, :], po_ps[:])
            po_ct += 1

        def po_store(postg_t, grow, oc, f0):
            nc.sync.dma_start(
                out_d[grow + f0 * 128:grow + (f0 + 4) * 128,
                      oc * SQ:(oc + 1) * SQ]
                .rearrange("(f p) t -> p f t", p=128),
                postg_t[:])

        def pop_pend(n, reserve=0):
            for _ in range(n):
                if len(pend) <= reserve:
                    return
                pend.pop(0)[2]()

        def pop_stale_norms(cur_blk):
            # emit any previous block's norm_muls (their sums chains are long
            # done) before this block's first write to the bufs=1 yty slot
            while any(k == "norm" and bl < cur_blk for k, bl, _ in pend):
                pend.pop(0)[2]()

        # the last pass-1 chunk's V transposes become the first PE filler
        # items (pass-2 j=0 has no out-proj work yet); evictions alternate
        # VectorE/ScalarE so the exp stream isn't delayed on ACT.
        def v_tr_one(vraw_p, gcol_p, t4, on_vec):
            nonlocal tp_alt
            tp = psum.tile([128, 128], f32, tag=["po0", "po1"][tp_alt],
                           name=f"tpf_{gcol_p}_{t4}")
            tp_alt ^= 1
            nc.tensor.transpose(
                tp[:], vraw_p[:, t4 * 128:(t4 + 1) * 128], idn_sb[:])
            dst = vbufT[:, gcol_p + t4 * 128:gcol_p + (t4 + 1) * 128]
            if on_vec:
                nc.vector.tensor_copy(dst, tp[:])
            else:
                nc.scalar.copy(dst, tp[:])
        while pending_v:
            vraw_p, gcol_p = pending_v.pop(0)
            for t4 in range(4):
                pend.append(("po", -1,
                             lambda v=vraw_p, g=gcol_p, t=t4, ov=(t4 % 2 == 0):
                             v_tr_one(v, g, t, ov)))

        def norm_mul(yts_t, yty_t, rr0_t, jj_, pr_, i_):
            h = pr_ * 2 + i_
            sidx = jj_ * 4 + h
            rb = spool.tile([128, SQ], bf16, tag="rb", bufs=2,
                            name=f"rb_{sidx}")
            nc.gpsimd.partition_broadcast(
                rb[:], rr0_t[0:1, i_, :], channels=128)
            nc.vector.tensor_mul(
                yts_t[:, h, jj_ * SQ:(jj_ + 1) * SQ],
                yty_t[:, sidx, :], rb[:])

        blk_ct = [0]

        for b in range(B):
            # process j-groups densest-first (j=3..0): the first group runs
            # with an empty filler backlog, so give it the best PE/exp ratio;
            # j=0 (mostly-masked, thin PE) runs last against a deep backlog.
            for hh in (1, 0):
                blk = blk_ct[0]
                blk_ct[0] += 1
                need_norm_drain = [True]
                yts = ypool.tile([128, HPC, HB], bf16, tag="yts",
                                 name=f"yts_{b}_{hh}")
                # reuse the (pass-1-only) wkv weight slot for the
                # unnormalized attention staging buffer: same pool tag ->
                # same SBUF bytes, WAR deps order it after the last QKV read.
                yty = wpool.tile([128, 8, SQ], bf16, tag="wkv", bufs=1,
                                 name=f"yty_{b}_{hh}")
                for jj in (1, 0):
                    j = hh * 2 + jj
                    last_jj = (b == 1 and hh == 0 and jj == 0)
                    gcol = b * T + j * SQ
                    nkc = 4 * j + 4
                    for pr in range(2):          # head pairs (2h per pass)
                        hs = (2 * pr, 2 * pr + 1)
                        yt_ps = [psum.tile([128, SQ], f32,
                                           tag=["yt0", "yt1"][i],
                                           name=f"yt_{b}_{j}_{h}")
                                 for i, h in enumerate(hs)]
                        acc2 = accp.tile([128, 1024], bf16, tag="acc",
                                         name=f"acc2_{b}_{j}_{pr}")
                        prev = None

                        def emit_av(kc, es_t, off):
                            st, sp = (kc == 0), (kc == nkc - 1)
                            koff = b * T + kc * 128
                            for i in range(2):
                                nc.tensor.matmul(
                                    yt_ps[i][:, off:SQ],
                                    vbufT[:, koff:koff + 128],
                                    es_t[:, i * SQ + off:(i + 1) * SQ],
                                    start=st, stop=sp)

                        for kc in range(nkc):
                            koff = b * T + kc * 128
                            off = 128 * max(0, kc - 4 * j)
                            s2 = psum.tile([128, 1024], f32,
                                           tag=sc_tag[kc % 2],
                                           name=f"s_{b}_{j}_{kc}_{pr}")
                            for i, h in enumerate(hs):
                                nc.tensor.matmul(
                                    s2[:, i * SQ + off:(i + 1) * SQ],
                                    rotK[:, koff:koff + 128],
                                    qrot[:, h, gcol + off:gcol + SQ],
                                    start=True, stop=True)
                            es = espool.tile([128, 1024], bf16, tag="es",
                                             name=f"es_{b}_{j}_{kc}_{pr}")
                            nc.scalar.activation(es[:], s2[:],
                                                 AF.Exp, scale=INV_SQRT_D)
                            if off > 0 or kc == 4 * j:
                                # mask the leading 128-wide causal triangle
                                # (DVE, NOT gpsimd: mixing ops on gpsimd
                                # thrashes its loadable Q7 library against
                                # partition_broadcast, ~6us per swap)
                                for i in range(2):
                                    nc.vector.tensor_mul(
                                        es[:, i * SQ + off:i * SQ + off + 128],
                                        es[:, i * SQ + off:i * SQ + off + 128],
                                        tri_sb[:])
                            # softmax denominators: accumulate exp on DVE
                            if kc == 0:
                                nc.vector.tensor_copy(acc2[:], es[:])
                            elif off == 0:
                                nc.vector.tensor_add(acc2[:], acc2[:], es[:])
                            else:
                                for i in range(2):
                                    nc.vector.tensor_add(
                                        acc2[:, i * SQ + off:(i + 1) * SQ],
                                        acc2[:, i * SQ + off:(i + 1) * SQ],
                                        es[:, i * SQ + off:(i + 1) * SQ])
                            if prev is not None:
                                emit_av(kc - 1, *prev)
                            prev = (es, off)
                            pop_pend(2, reserve=20)
                        emit_av(nkc - 1, *prev)

                        if need_norm_drain[0]:
                            pop_stale_norms(blk)
                            need_norm_drain[0] = False
                        # stage attention outputs out of PSUM right away
                        for i, h in enumerate(hs):
                            sidx = jj * 4 + pr * 2 + i
                            nc.vector.tensor_copy(yty[:, sidx, :],
                                                  yt_ps[i][:])
                        pop_pend(2)
                        # per-pair softmax sums: 2 selector matmuls reduce the
                        # bf16 accumulator's 128 key-residues; the whole
                        # reciprocal chain stays on DVE (no ACT-queue hops)
                        sums2 = psum.tile([2, SQ], f32, tag="yt0",
                                          name=f"sums2_{b}_{j}_{pr}")
                        for i in range(2):
                            nc.tensor.matmul(
                                sums2[:], sel_sb[:, 4 * i:4 * i + 2],
                                acc2[:, i * SQ:(i + 1) * SQ],
                                start=(i == 0), stop=(i == 1))
                        stg = spool.tile([2, SQ], f32, tag="vraw", bufs=2,
                                         name=f"stg_{b}_{j}_{pr}")
                        nc.vector.tensor_copy(stg[:], sums2[:])
                        rrf = spool.tile([2, SQ], f32, tag="vraw", bufs=2,
                                         name=f"rrf_{b}_{j}_{pr}")
                        nc.vector.reciprocal_approx_fast(rrf[:], stg[:])
                        rrb = spool.tile([2, SQ], bf16, tag="rrb", bufs=2,
                                         name=f"rrb_{b}_{j}_{pr}")
                        with nc.allow_low_precision(reason="softmax recip"):
                            nc.vector.tensor_copy(rrb[:], rrf[:])
                        # partition_broadcast only reads partition 0: hop the
                        # rows there via tiny SBUF->SBUF DMAs (idle sync ring)
                        rr0 = spool.tile([1, 2, SQ], bf16, tag="rr0", bufs=2,
                                         name=f"rr0_{b}_{j}_{pr}")
                        for i in range(2):
                            nc.sync.dma_start(rr0[0:1, i, :], rrb[i:i + 1, :])
                        for i in range(2):
                            pend.append(("norm", blk,
                                         lambda y=yts, yy=yty, dd=rr0,
                                         a=jj, p=pr, q=i:
                                         norm_mul(y, yy, dd, a, p, q)))

                    grow = b * T + hh * HB
                    for oc in range(8):
                        postg = ppool.tile([128, 4, SQ], f16, tag="po",
                                           name=f"postg_{b}_{hh}_{jj}_{oc}")
                        for k4, ts8 in enumerate(range(jj * 4, jj * 4 + 4)):
                            pend.append(("po", blk,
                                         lambda y=yts, g=grow, o=oc, t=ts8,
                                         pt=postg, k=k4:
                                         po_group(y, g, o, t, pt, k)))
                        pend.append(("po", blk,
                                     lambda pt=postg, g=grow, o=oc, f0=jj * 4:
                                     po_store(pt, g, o, f0)))
        drain[0] = True
        pop_pend(len(pend))

    nc.compile()
    return nc


_PROG = None


def kernel(x, freq_cos, freq_sin, w_q_w, w_q_b, w_kv_w, w_kv_b, proj_w, proj_b,
           start_pos=0, **_unused):
    global _PROG
    import ml_dtypes
    from concourse.bass_utils import run_bass_kernel_spmd

    bf16 = ml_dtypes.bfloat16

    x = np.asarray(x, np.float32)
    freq_cos = np.asarray(freq_cos, np.float32)
    freq_sin = np.asarray(freq_sin, np.float32)
    w_q_w = np.asarray(w_q_w, np.float32)
    w_q_b = np.asarray(w_q_b, np.float32)
    w_kv_w = np.asarray(w_kv_w, np.float32)
    w_kv_b = np.asarray(w_kv_b, np.float32)
    proj_w = np.asarray(proj_w, np.float32)
    proj_b = np.asarray(proj_b, np.float32)

    xT = np.ascontiguousarray(x.reshape(BT, E).T).astype(bf16)

    cosE = np.repeat(freq_cos.T, 2, axis=0).astype(np.float32)        # [128, T]
    sinE = np.repeat(freq_sin.T, 2, axis=0).astype(np.float32)
    sinS = sinE.copy()
    sinS[0::2, :] *= -1.0                                             # even rows -sin
    cosE = cosE.astype(bf16)
    sinS = sinS.astype(bf16)

    kp = np.arange(128)[:, None]
    qq = np.arange(128)[None, :]
    triM = (qq >= kp).astype(bf16)                                    # [128, 128]

    sel16 = np.zeros((128, 16), np.float32)
    for h in range(4):
        sel16[:, 4 * h + h] = 1.0
    sel16 = sel16.astype(bf16)

    ident = np.eye(128, dtype=np.float32)

    if _PROG is None:
        _PROG = _build_program()

    in_maps = []
    for c in range(NCORES):
        wq_c = np.ascontiguousarray(
            w_q_w[c * 512:(c + 1) * 512, :].T).astype(bf16)            # [E, 512]
        kT = w_kv_w[c * D:(c + 1) * D, :].T                            # [E, 128]
        vT = w_kv_w[8 * D + c * D:8 * D + (c + 1) * D, :].T
        wkv_c = np.ascontiguousarray(
            np.concatenate([kT, vT], axis=1)).astype(bf16)             # [E, 256]
        biases = np.zeros((6, 128), np.float32)
        biases[0:4, :] = w_q_b[c * 512:(c + 1) * 512].reshape(4, 128)
        biases[4, :] = w_kv_b[c * D:(c + 1) * D]
        biases[5, :] = w_kv_b[8 * D + c * D:8 * D + (c + 1) * D]
        pjt_c = np.ascontiguousarray(
            proj_w[:, c * 512:(c + 1) * 512].T).astype(bf16)           # [512, E]
        in_maps.append({
            "xT": xT, "wqT": wq_c, "wkvT": wkv_c, "biases": biases,
            "cosE": cosE, "sinS": sinS, "triM": triM, "sel16": sel16,
            "projT": pjt_c, "ident": ident,
        })

    res = run_bass_kernel_spmd(_PROG, in_maps, core_ids=list(range(NCORES)))
    out = np.zeros((BT, E), np.float32)
    for c in range(NCORES):
        out += res.results[c]["yp"].astype(np.float32)
    out = out + proj_b[None, :].astype(np.float32)
    return out.reshape(B, T, E).astype(np.float32)
